# revision 32
# baseline (speedup 1.0000x reference)
"""Trainium2 Bass kernel for nn_ActionModel (2x GINEConv + mean-pool + MLP head).

Strategy (8 NeuronCores, SPMD):
  - Nodes sharded by graph: core m owns 8 consecutive graphs = 8192 nodes.
  - Edges sharded by dst owner; per core, edges are grouped by 128-dst block,
    padded to a fixed per-block capacity C so the instruction stream is
    identical across cores.
  - Host prep builds, per core, sequentially-streamable operand arrays in
    padded edge order (the same treatment the edge_attr already gets):
      * xg  : x[src]+be (bf16) laid out [128 lanes, chunk, feat]
      * eaT4: edge_attr 4-phase packed so one K=128 matmul against a
              block-diagonal We computes ea@We for 4 chunks at once
      * dstcol: per-edge dst-local-in-block (bf16, 128 = padding sentinel)
  - On-device, per 1024-edge pair of 4-chunk groups:
      TensorE: ea@We (one N=512 matmul per group) + identity-matmul add of
      xg into PSUM; ACT applies ReLU over [128,1024] -> bf16 msg; DVE builds
      the dst one-hot S per 128-dst block (iota/is_equal); TensorE
      accumulates aggT += msg^T @ S into [feat, dst] PSUM.
  - Node stage: yT = aggT + xT; Linear+folded-BN+ReLU via TensorE/ACT.
  - Two launches: L1 -> hT (bf16); host rebuilds the conv2 edge stream
    (h+be2)[src]; L2 runs conv2, sigmoid with per-block accum_out giving
    block sums, per-graph mean pool (graphs are contiguous 1024-node
    ranges), and the 3-layer head. Only [A, GPC] per core comes back.
"""

import heapq
import os
import sys
import numpy as np

for _p in ("/opt/trn_rl_repo",):
    if _p not in sys.path and os.path.isdir(_p):
        sys.path.insert(0, _p)

import ml_dtypes  # noqa: E402

BF16 = ml_dtypes.bfloat16
F8 = ml_dtypes.float8_e4m3


def _enable_ldw_opt():
    """Flip walrus's --enable-ldw-opt to true (merges/accelerates redundant
    LDWEIGHTS). Wraps concourse.bass_utils.run_command."""
    # walrus rejects bass-emitted InstLdweights under ldw-opt; keep off
    # unless explicitly requested for experiments.
    if not os.environ.get("BASS_GNN_LDWOPT"):
        return
    from concourse import bass_utils as _bu
    if getattr(_bu, "_gnn_ldwopt_patched", False):
        return
    _orig = _bu.run_command

    def _patched(cmd, *a, **k):
        if isinstance(cmd, list):
            cmd = ["--enable-ldw-opt=true" if c == "--enable-ldw-opt=false"
                   else c for c in cmd]
        return _orig(cmd, *a, **k)

    _bu.run_command = _patched
    _bu._gnn_ldwopt_patched = True

# ---------------------------------------------------------------- config ----

class Cfg:
    def __init__(self, N=65536, E=1048576, H=128, FE=32, NG=64, A=32,
                 n_cores=8, WBLK=4, bn_eps=1e-5):
        self.N, self.E, self.H, self.FE, self.NG, self.A = N, E, H, FE, NG, A
        self.n_cores = n_cores
        self.WBLK = WBLK          # dst blocks per window
        self.bn_eps = bn_eps
        self.NPC = N // n_cores   # nodes per core
        self.GPC = NG // n_cores  # graphs per core
        self.NBLK = self.NPC // 128
        assert self.NPC % 128 == 0 and self.NBLK % WBLK == 0
        self.NW = self.NBLK // WBLK
        self.C = None             # per-block capacity; set by prep

    @property
    def CPB(self):  # chunks per block
        return self.C // 128

    @property
    def CPW(self):  # chunks per window
        return self.WBLK * self.CPB

    @property
    def EPW(self):  # padded edge positions per window
        return self.CPW * 128

    @property
    def EP(self):   # padded edge positions per core
        return self.NBLK * self.C


# ------------------------------------------------------------- host prep ----

def host_prep(cfg, x, edge_index, edge_attr, batch,
              We1, be1, W1, b1, g1, bt1, m1, v1,
              We2, be2, W2, b2, g2, bt2, m2, v2,
              Wa1, ba1, ga1, bta1, ma1, va1,
              Wa2, ba2, ga2, bta2, ma2, va2,
              Wa3, ba3):
    """Partition/sort/pad edges, build per-core streamable arrays."""
    N, H, NC = cfg.N, cfg.H, cfg.n_cores
    NPC, NBLK = cfg.NPC, cfg.NBLK

    src = np.asarray(edge_index[0], dtype=np.int64)
    dst = np.asarray(edge_index[1], dtype=np.int64)
    batch = np.asarray(batch, dtype=np.int64)
    x = np.asarray(x, dtype=np.float32)
    edge_attr = np.asarray(edge_attr, dtype=np.float32)

    cnts = np.bincount(batch, minlength=cfg.NG)
    assert (cnts == cfg.N // cfg.NG).all(), "equal-size graphs expected"

    # Within-graph node relabeling balancing per-block in-degree (greedy
    # first-fit-decreasing into the 8 blocks of each graph). Shrinks the
    # padded per-block capacity C. Pooling is within-graph permutation
    # invariant; the gather table stays in original node ids.
    GS = N // cfg.NG
    BPG = GS // 128
    indeg = np.bincount(dst, minlength=N)
    newpos = np.empty(N, np.int64)
    for g in range(cfg.NG):
        deg = indeg[g * GS:(g + 1) * GS]
        order_g = np.argsort(-deg, kind="stable")
        heap = [(0, 0, b) for b in range(BPG)]
        heapq.heapify(heap)
        slot = np.empty(GS, np.int64)
        for nd in order_g:
            load, c, b = heapq.heappop(heap)
            slot[nd] = b * 128 + c
            load += int(deg[nd])
            c += 1
            if c < 128:
                heapq.heappush(heap, (load, c, b))
        newpos[g * GS:(g + 1) * GS] = g * GS + slot
    invp = np.argsort(newpos)
    assert (batch[invp] == batch).all()
    dstp = newpos[dst]

    core = dstp // NPC
    local = dstp - core * NPC
    blk = local >> 7
    dl = local & 127

    seg = core * NBLK + blk
    n_seg = NC * NBLK
    order = np.lexsort((src, seg))
    seg_o = seg[order]
    seg_cnt = np.bincount(seg_o, minlength=n_seg)
    C = int(np.max(seg_cnt))
    C = max(128, -(-C // 128) * 128)
    cfg.C = C
    EP = cfg.EP

    seg_start = np.zeros(n_seg, np.int64)
    np.cumsum(seg_cnt[:-1], out=seg_start[1:])
    within = np.arange(len(order)) - seg_start[seg_o]
    pos = (seg_o % NBLK) * C + within          # core-relative padded pos
    core_o = seg_o // NBLK

    src_at = np.zeros((NC, EP), np.int64)
    src_at[core_o, pos] = src[order]
    dstl_at = np.full((NC, EP), 128.0, np.float32)
    dstl_at[core_o, pos] = dl[order].astype(np.float32)
    ea_at = np.zeros((NC, EP, cfg.FE), np.float32)
    ea_at[core_o, pos] = edge_attr[order]

    # eaT4: 4-phase layout. Edge position p (chunk c=p//128, lane e=p%128)
    # maps to [32*(c%4)+f, (c//4)*128+e] — each 128-col block is a shared
    # K=128 matmul lhsT covering 4 chunks (phase selection via the
    # block-diagonal We).
    G4 = EP // 512
    eaT4 = ea_at.reshape(NC, G4, 4, 128, cfg.FE).transpose(0, 2, 4, 1, 3) \
        .reshape(NC, 4 * cfg.FE, G4 * 128).astype(BF16)

    dstcol = dstl_at.reshape(NC, EP // 128, 128).transpose(0, 2, 1) \
        .astype(BF16).copy()

    # node-side arrays (new node order)
    xT = x[invp].reshape(NC, NPC, H).transpose(0, 2, 1) \
        .astype(np.float32).copy()

    f32 = lambda a: np.asarray(a, np.float32)
    xtab = (x + f32(be1)[None, :]).astype(F8)

    def bnfold(g, bt, m, v, b):
        A_ = f32(g) / np.sqrt(f32(v) + cfg.bn_eps)
        B_ = A_ * f32(b) + (f32(bt) - A_ * f32(m))
        return A_.reshape(-1, 1), B_.reshape(-1, 1)

    A1, B1 = bnfold(g1, bt1, m1, v1, b1)
    A2, B2 = bnfold(g2, bt2, m2, v2, b2)
    Aa1, Ba1 = bnfold(ga1, bta1, ma1, va1, ba1)
    Aa2, Ba2 = bnfold(ga2, bta2, ma2, va2, ba2)

    def wsel(We_):  # [128, 4*H]: block q has We at rows 32q..32q+31
        W_ = np.zeros((128, 4 * H), np.float32)
        for q in range(4):
            W_[32 * q:32 * q + cfg.FE, q * H:(q + 1) * H] = f32(We_)
        return W_.astype(BF16)

    wts = dict(
        We1=wsel(We1),
        We2=wsel(We2),
        W1=f32(W1).astype(BF16), W2=f32(W2).astype(BF16),
        A1=A1, B1=B1, A2=A2, B2=B2,
        be2=f32(be2),
        # mean pool (1/1024) folded into Wa1
        Wa1=f32(Wa1) / (cfg.N // cfg.NG), Aa1=Aa1, Ba1=Ba1,
        Wa2=f32(Wa2), Aa2=Aa2, Ba2=Ba2,
        Wa3=f32(Wa3), ba3=f32(ba3).reshape(-1, 1),
    )
    percore = dict(eaT4=eaT4, dstcol=dstcol, xT=xT, src_at=src_at,
                   newpos=newpos)
    return xtab, percore, wts


def pack_stream(tab, src_at, EP):
    """tab [N, 128] bf16, src_at [NC, EP] -> [NC, 128, EP] bf16 where
    out[c, lane, ch*128+f] = tab[src_at[c, ch*128+lane], f]."""
    NC = src_at.shape[0]
    g = tab[src_at.reshape(-1)]                    # [NC*EP, 128]
    g = g.reshape(NC, EP // 128, 128, 128)         # [c, ch, lane, f]
    return np.ascontiguousarray(g.transpose(0, 2, 1, 3)).reshape(NC, 128, EP)


# --------------------------------------------------------- bass programs ----

def build_program(cfg, launch):
    """launch: 1 (conv1 -> h) or 2 (conv2 + pool + head)."""
    import concourse.bacc as bacc
    import concourse.tile as tile
    from concourse import mybir
    from concourse.masks import make_identity

    dt = mybir.dt
    AF = mybir.ActivationFunctionType
    OP = mybir.AluOpType
    H = cfg.H
    NPC, NBLK, WBLK, NW = cfg.NPC, cfg.NBLK, cfg.WBLK, cfg.NW
    C, CPB, CPW, EPW, EP = cfg.C, cfg.CPB, cfg.CPW, cfg.EPW, cfg.EP
    assert CPW % 4 == 0
    NG4 = CPW // 4
    # batches of 1-2 four-chunk groups sharing one PSUM tile / ACT
    batches = [(2 * i, 2 * i + 1) for i in range(NG4 // 2)]
    if NG4 % 2:
        batches.append((NG4 - 1,))

    nc = bacc.Bacc("TRN2", target_bir_lowering=False, debug=False,
                   enable_asserts=False, num_devices=cfg.n_cores)

    din = lambda n, s, d: nc.dram_tensor(n, s, d, kind="ExternalInput").ap()
    dout = lambda n, s, d: nc.dram_tensor(n, s, d, kind="ExternalOutput").ap()

    xg = din("xg", [128, EP], dt.float8e4)
    eaT4 = din("eaT4", [128, EP // 4], dt.bfloat16)
    dstcol = din("dstcol", [128, EP // 128], dt.bfloat16)
    We = din("We", [128, 4 * H], dt.bfloat16)
    W = din("W", [H, H], dt.bfloat16)
    Asc = din("Asc", [H, 1], dt.float32)
    Bsc = din("Bsc", [H, 1], dt.float32)
    if launch == 1:
        xT = din("xT", [128, NPC], dt.float32)
        hT_out = dout("hT_out", [128, NPC], dt.bfloat16)
    else:
        xT = din("xT", [128, NPC], dt.bfloat16)
        Wa1 = din("Wa1", [H, H], dt.float32)
        Aa1 = din("Aa1", [H, 1], dt.float32)
        Ba1 = din("Ba1", [H, 1], dt.float32)
        Wa2 = din("Wa2", [H, H], dt.float32)
        Aa2 = din("Aa2", [H, 1], dt.float32)
        Ba2 = din("Ba2", [H, 1], dt.float32)
        Wa3 = din("Wa3", [H, cfg.A], dt.float32)
        ba3 = din("ba3", [cfg.A, 1], dt.float32)
        act_out = dout("act_out", [cfg.A, cfg.GPC], dt.float32)

    with tile.TileContext(nc) as tc:
        with (
            tc.tile_pool(name="const", bufs=1) as cpool,
            tc.tile_pool(name="xg", bufs=2) as xgpool,
            tc.tile_pool(name="stream", bufs=2) as spool,
            tc.tile_pool(name="sS", bufs=2) as spool_S,
            tc.tile_pool(name="work", bufs=3) as wpool,
            tc.tile_pool(name="blk", bufs=3) as bpool,
            tc.tile_pool(name="ps_t", bufs=2, space="PSUM") as ps_t,
            tc.tile_pool(name="ps_agg", bufs=2, space="PSUM") as ps_agg,
            tc.tile_pool(name="ps_misc", bufs=2, space="PSUM") as ps_misc,
        ):
            # ---- persistent constants
            dstcol_sb = cpool.tile([128, EP // 128], dt.bfloat16, tag="dstc")
            We_sb = cpool.tile([128, 4 * H], dt.bfloat16, tag="We")
            W_sb = cpool.tile([H, H], dt.bfloat16, tag="W")
            A_sb = cpool.tile([H, 1], dt.float32, tag="Asc")
            B_sb = cpool.tile([H, 1], dt.float32, tag="Bsc")
            nc.sync.dma_start(dstcol_sb[:], dstcol[:])
            nc.sync.dma_start(We_sb[:], We[:])
            nc.sync.dma_start(W_sb[:], W[:])
            nc.sync.dma_start(A_sb[:], Asc[:])
            nc.sync.dma_start(B_sb[:], Bsc[:])

            iota_sb = cpool.tile([128, 128], dt.bfloat16, tag="iota")
            nc.gpsimd.iota(iota_sb[:], pattern=[[1, 128]], base=0,
                           channel_multiplier=0,
                           allow_small_or_imprecise_dtypes=True)
            id_f8 = cpool.tile([128, 128], dt.float8e4, tag="idf8")
            make_identity(nc, id_f8[:])

            if launch == 2:
                Wa1_sb = cpool.tile([H, H], dt.float32, tag="Wa1")
                Wa2_sb = cpool.tile([H, H], dt.float32, tag="Wa2")
                Wa3_sb = cpool.tile([H, cfg.A], dt.float32, tag="Wa3")
                Aa1_sb = cpool.tile([H, 1], dt.float32, tag="Aa1")
                Ba1_sb = cpool.tile([H, 1], dt.float32, tag="Ba1")
                Aa2_sb = cpool.tile([H, 1], dt.float32, tag="Aa2")
                Ba2_sb = cpool.tile([H, 1], dt.float32, tag="Ba2")
                ba3_sb = cpool.tile([cfg.A, 1], dt.float32, tag="ba3")
                for a, b in ((Wa1_sb, Wa1), (Wa2_sb, Wa2), (Wa3_sb, Wa3),
                             (Aa1_sb, Aa1), (Ba1_sb, Ba1), (Aa2_sb, Aa2),
                             (Ba2_sb, Ba2), (ba3_sb, ba3)):
                    nc.sync.dma_start(a[:], b[:])
                bs_sb = cpool.tile([128, NBLK], dt.float32, tag="bs")

            # ---- main loop over windows
            for wdx in range(NW):
                xg_sl = xgpool.tile([128, EPW], dt.float8e4, tag="xg")
                nc.sync.dma_start(xg_sl[:],
                                  xg[:, wdx * EPW:(wdx + 1) * EPW])
                ea_sl = spool.tile([128, EPW // 4], dt.bfloat16, tag="ea")
                nc.sync.dma_start(
                    ea_sl[:], eaT4[:, wdx * (EPW // 4):(wdx + 1) * (EPW // 4)])
                xt_sl = spool.tile([128, WBLK * 128],
                                   dt.float32 if launch == 1 else dt.bfloat16,
                                   tag="xt")
                nc.sync.dma_start(xt_sl[:],
                                  xT[:, wdx * WBLK * 128:(wdx + 1) * WBLK * 128])

                # dst one-hot S per 128-dst block (CPB chunks each)
                S_blk = []
                for bw in range(WBLK):
                    c0 = wdx * CPW + bw * CPB
                    S_b = spool_S.tile([128, CPB, 128], dt.bfloat16,
                                       tag=f"S{bw}")
                    nc.vector.tensor_tensor(
                        out=S_b[:],
                        in0=iota_sb[:].unsqueeze(1)
                            .to_broadcast([128, CPB, 128]),
                        in1=dstcol_sb[:, c0:c0 + CPB].unsqueeze(2)
                            .to_broadcast([128, CPB, 128]),
                        op=OP.is_equal)
                    S_blk.append(S_b)

                agg_ps = ps_agg.tile([128, WBLK * 128], dt.float32, tag="agg")

                for grp in batches:
                    nw = 512 * len(grp)
                    t_ps = ps_t.tile([128, 1024], dt.float32, tag="t")
                    for gi, Gw in enumerate(grp):
                        lhs = ea_sl[:, Gw * 128:(Gw + 1) * 128]
                        nc.tensor.matmul(t_ps[:, gi * 512:(gi + 1) * 512],
                                         lhsT=lhs, rhs=We_sb[:],
                                         start=True, stop=False,
                                         skip_group_check=True)
                    for gi, Gw in enumerate(grp):
                        nc.tensor.matmul(t_ps[:, gi * 512:(gi + 1) * 512],
                                         lhsT=id_f8[:],
                                         rhs=xg_sl[:, Gw * 512:(Gw + 1) * 512],
                                         start=False, stop=True,
                                         skip_group_check=True)
                    msg = wpool.tile([128, 1024], dt.bfloat16, tag="msg")
                    nc.scalar.activation(msg[:, 0:nw], t_ps[:, 0:nw], AF.Relu)
                    for j in range(4 * len(grp)):
                        ch = grp[0] * 4 + j
                        bw, ci = divmod(ch, CPB)
                        nc.tensor.matmul(
                            agg_ps[:, bw * 128:(bw + 1) * 128],
                            lhsT=msg[:, j * 128:(j + 1) * 128],
                            rhs=S_blk[bw][:, ci, :],
                            start=(ci == 0), stop=(ci == CPB - 1),
                            skip_group_check=True)

                # drain: yT = aggT + xT, then Linear+BN(+act) per block
                yT = wpool.tile([128, WBLK * 128], dt.bfloat16, tag="yT")
                nc.vector.tensor_tensor(out=yT[:], in0=agg_ps[:], in1=xt_sl[:],
                                        op=OP.add)
                for k in range(WBLK):
                    b_abs = wdx * WBLK + k
                    hp_ps = ps_misc.tile([128, 128], dt.float32, tag="m")
                    nc.tensor.matmul(hp_ps[:], lhsT=W_sb[:],
                                     rhs=yT[:, k * 128:(k + 1) * 128],
                                     start=True, stop=True,
                                     skip_group_check=True)
                    if launch == 1:
                        hT_t = bpool.tile([128, 128], dt.bfloat16, tag="hT")
                        nc.scalar.activation(hT_t[:], hp_ps[:], AF.Relu,
                                             bias=B_sb[:], scale=A_sb[:])
                        nc.sync.dma_start(
                            hT_out[:, b_abs * 128:(b_abs + 1) * 128], hT_t[:])
                    else:
                        # sigmoid(relu(z)) == max(sigmoid(z), 0.5)
                        sT = bpool.tile([128, 128], dt.float32, tag="sT")
                        nc.scalar.activation(sT[:], hp_ps[:], AF.Sigmoid,
                                             bias=B_sb[:], scale=A_sb[:])
                        h2T = bpool.tile([128, 128], dt.bfloat16, tag="h2T")
                        nc.vector.tensor_scalar(
                            out=h2T[:], in0=sT[:], scalar1=0.5, scalar2=0.0,
                            op0=OP.max, op1=OP.add,
                            accum_out=bs_sb[:, b_abs:b_abs + 1])

            if launch == 2:
                # per-graph sums (graphs are 8 consecutive blocks), head
                pooledT = bpool.tile([128, cfg.GPC], dt.float32, tag="plT")
                for g in range(cfg.GPC):
                    nc.vector.tensor_reduce(
                        out=pooledT[:, g:g + 1],
                        in_=bs_sb[:, g * 8:(g + 1) * 8],
                        axis=mybir.AxisListType.X, op=OP.add)

                a1_ps = ps_misc.tile([128, cfg.GPC], dt.float32, tag="m")
                nc.tensor.matmul(a1_ps[:], lhsT=Wa1_sb[:], rhs=pooledT[:],
                                 start=True, stop=True, skip_group_check=True)
                a1 = bpool.tile([128, cfg.GPC], dt.float32, tag="a1")
                nc.scalar.activation(a1[:], a1_ps[:], AF.Relu,
                                     bias=Ba1_sb[:], scale=Aa1_sb[:])
                a2_ps = ps_misc.tile([128, cfg.GPC], dt.float32, tag="m")
                nc.tensor.matmul(a2_ps[:], lhsT=Wa2_sb[:], rhs=a1[:],
                                 start=True, stop=True, skip_group_check=True)
                a2 = bpool.tile([128, cfg.GPC], dt.float32, tag="a2")
                nc.scalar.activation(a2[:], a2_ps[:], AF.Relu,
                                     bias=Ba2_sb[:], scale=Aa2_sb[:])
                a3_ps = ps_misc.tile([cfg.A, cfg.GPC], dt.float32, tag="m")
                nc.tensor.matmul(a3_ps[:], lhsT=Wa3_sb[:], rhs=a2[:],
                                 start=True, stop=True, skip_group_check=True)
                a3 = bpool.tile([cfg.A, cfg.GPC], dt.float32, tag="a3")
                nc.scalar.activation(a3[:], a3_ps[:], AF.Sigmoid,
                                     bias=ba3_sb[:])
                nc.sync.dma_start(act_out[:], a3[:])

    nc.compile()
    return nc


# ------------------------------------------------------------- execution ----

def make_in_maps(cfg, launch, xg_pc, percore, wts, hT_percore=None):
    NC = cfg.n_cores
    maps = []
    for c in range(NC):
        m = dict(xg=np.ascontiguousarray(xg_pc[c]),
                 eaT4=np.ascontiguousarray(percore["eaT4"][c]),
                 dstcol=np.ascontiguousarray(percore["dstcol"][c]))
        if launch == 1:
            m.update(xT=np.ascontiguousarray(percore["xT"][c]),
                     We=wts["We1"], W=wts["W1"], Asc=wts["A1"], Bsc=wts["B1"])
        else:
            m.update(xT=np.ascontiguousarray(hT_percore[c]),
                     We=wts["We2"], W=wts["W2"], Asc=wts["A2"], Bsc=wts["B2"],
                     Wa1=wts["Wa1"], Aa1=wts["Aa1"], Ba1=wts["Ba1"],
                     Wa2=wts["Wa2"], Aa2=wts["Aa2"], Ba2=wts["Ba2"],
                     Wa3=wts["Wa3"], ba3=wts["ba3"])
        maps.append(m)
    return maps


_PROG_CACHE = {}
LAST_EXEC_NS = {}


def kernel(**inputs):
    from concourse import bass_utils
    _enable_ldw_opt()

    cfg = Cfg()
    xtab, percore, wts = host_prep(cfg, **inputs)

    key = (cfg.N, cfg.E, cfg.C)
    if key not in _PROG_CACHE:
        _PROG_CACHE[key] = (build_program(cfg, 1), build_program(cfg, 2))
    nc1, nc2 = _PROG_CACHE[key]

    trace = bool(int(os.environ.get("BASS_GNN_TRACE", "0")))
    core_ids = list(range(cfg.n_cores))

    xg1 = pack_stream(xtab, percore["src_at"], cfg.EP)
    maps1 = make_in_maps(cfg, 1, xg1, percore, wts)
    res1 = bass_utils.run_bass_kernel_spmd(nc1, maps1, core_ids=core_ids,
                                           trace=trace)
    LAST_EXEC_NS["L1"] = res1.exec_time_ns
    if os.environ.get("BASS_GNN_ONLY_L1"):
        return res1
    hT = [res1.results[c]["hT_out"] for c in core_ids]      # [128, NPC] bf16

    h_all = np.concatenate([t.T for t in hT], axis=0)       # [N, H] new order
    h_orig = h_all[percore["newpos"]]                       # rows by orig id
    htab = (h_orig.astype(np.float32) + wts["be2"][None, :]).astype(F8)
    xg2 = pack_stream(htab, percore["src_at"], cfg.EP)

    maps2 = make_in_maps(cfg, 2, xg2, percore, wts, hT_percore=hT)
    res2 = bass_utils.run_bass_kernel_spmd(nc2, maps2, core_ids=core_ids,
                                           trace=trace)
    LAST_EXEC_NS["L2"] = res2.exec_time_ns

    out = np.zeros((cfg.NG, cfg.A), np.float32)
    for c in core_ids:
        a3 = res2.results[c]["act_out"]          # [A, GPC]
        out[c * cfg.GPC:(c + 1) * cfg.GPC, :] = a3.T
    return out


# revision 35
# speedup vs baseline: 1.0513x; 1.0513x over previous
"""Trainium2 Bass kernel for nn_ActionModel (2x GINEConv + mean-pool + MLP head).

Strategy (8 NeuronCores, SPMD):
  - Nodes sharded by graph: core m owns 8 consecutive graphs = 8192 nodes.
  - Edges sharded by dst owner; per core, edges are grouped by 128-dst block,
    padded to a fixed per-block capacity C so the instruction stream is
    identical across cores.
  - Host prep builds, per core, sequentially-streamable operand arrays in
    padded edge order (the same treatment the edge_attr already gets):
      * xg  : x[src]+be (bf16) laid out [128 lanes, chunk, feat]
      * eaT4: edge_attr 4-phase packed so one K=128 matmul against a
              block-diagonal We computes ea@We for 4 chunks at once
      * dstcol: per-edge dst-local-in-block (bf16, 128 = padding sentinel)
  - On-device, per 1024-edge pair of 4-chunk groups:
      TensorE: ea@We (one N=512 matmul per group) + identity-matmul add of
      xg into PSUM; ACT applies ReLU over [128,1024] -> bf16 msg; DVE builds
      the dst one-hot S per 128-dst block (iota/is_equal); TensorE
      accumulates aggT += msg^T @ S into [feat, dst] PSUM.
  - Node stage: yT = aggT + xT; Linear+folded-BN+ReLU via TensorE/ACT.
  - Two launches: L1 -> hT (bf16); host rebuilds the conv2 edge stream
    (h+be2)[src]; L2 runs conv2, sigmoid with per-block accum_out giving
    block sums, per-graph mean pool (graphs are contiguous 1024-node
    ranges), and the 3-layer head. Only [A, GPC] per core comes back.
"""

import heapq
import os
import sys
import numpy as np

for _p in ("/opt/trn_rl_repo",):
    if _p not in sys.path and os.path.isdir(_p):
        sys.path.insert(0, _p)

import ml_dtypes  # noqa: E402

BF16 = ml_dtypes.bfloat16
F8 = ml_dtypes.float8_e4m3


def _enable_ldw_opt():
    """Flip walrus's --enable-ldw-opt to true (merges/accelerates redundant
    LDWEIGHTS). Wraps concourse.bass_utils.run_command."""
    # walrus rejects bass-emitted InstLdweights under ldw-opt; keep off
    # unless explicitly requested for experiments.
    if not os.environ.get("BASS_GNN_LDWOPT"):
        return
    from concourse import bass_utils as _bu
    if getattr(_bu, "_gnn_ldwopt_patched", False):
        return
    _orig = _bu.run_command

    def _patched(cmd, *a, **k):
        if isinstance(cmd, list):
            cmd = ["--enable-ldw-opt=true" if c == "--enable-ldw-opt=false"
                   else c for c in cmd]
        return _orig(cmd, *a, **k)

    _bu.run_command = _patched
    _bu._gnn_ldwopt_patched = True

# ---------------------------------------------------------------- config ----

class Cfg:
    def __init__(self, N=65536, E=1048576, H=128, FE=32, NG=64, A=32,
                 n_cores=8, WBLK=4, bn_eps=1e-5):
        self.N, self.E, self.H, self.FE, self.NG, self.A = N, E, H, FE, NG, A
        self.n_cores = n_cores
        self.WBLK = WBLK          # dst blocks per window
        self.bn_eps = bn_eps
        self.NPC = N // n_cores   # nodes per core
        self.GPC = NG // n_cores  # graphs per core
        self.NBLK = self.NPC // 128
        assert self.NPC % 128 == 0 and self.NBLK % WBLK == 0
        self.NW = self.NBLK // WBLK
        self.C = None             # per-block capacity; set by prep

    @property
    def CPB(self):  # chunks per block
        return self.C // 128

    @property
    def CPW(self):  # chunks per window
        return self.WBLK * self.CPB

    @property
    def EPW(self):  # padded edge positions per window
        return self.CPW * 128

    @property
    def EP(self):   # padded edge positions per core
        return self.NBLK * self.C


# ------------------------------------------------------------- host prep ----

def host_prep(cfg, x, edge_index, edge_attr, batch,
              We1, be1, W1, b1, g1, bt1, m1, v1,
              We2, be2, W2, b2, g2, bt2, m2, v2,
              Wa1, ba1, ga1, bta1, ma1, va1,
              Wa2, ba2, ga2, bta2, ma2, va2,
              Wa3, ba3):
    """Partition/sort/pad edges, build per-core streamable arrays."""
    N, H, NC = cfg.N, cfg.H, cfg.n_cores
    NPC, NBLK = cfg.NPC, cfg.NBLK

    src = np.asarray(edge_index[0], dtype=np.int64)
    dst = np.asarray(edge_index[1], dtype=np.int64)
    batch = np.asarray(batch, dtype=np.int64)
    x = np.asarray(x, dtype=np.float32)
    edge_attr = np.asarray(edge_attr, dtype=np.float32)

    cnts = np.bincount(batch, minlength=cfg.NG)
    assert (cnts == cfg.N // cfg.NG).all(), "equal-size graphs expected"

    # Within-graph node relabeling balancing per-block in-degree (greedy
    # first-fit-decreasing into the 8 blocks of each graph). Shrinks the
    # padded per-block capacity C. Pooling is within-graph permutation
    # invariant; the gather table stays in original node ids.
    GS = N // cfg.NG
    BPG = GS // 128
    indeg = np.bincount(dst, minlength=N)
    newpos = np.empty(N, np.int64)
    for g in range(cfg.NG):
        deg = indeg[g * GS:(g + 1) * GS]
        order_g = np.argsort(-deg, kind="stable")
        heap = [(0, 0, b) for b in range(BPG)]
        heapq.heapify(heap)
        slot = np.empty(GS, np.int64)
        for nd in order_g:
            load, c, b = heapq.heappop(heap)
            slot[nd] = b * 128 + c
            load += int(deg[nd])
            c += 1
            if c < 128:
                heapq.heappush(heap, (load, c, b))
        newpos[g * GS:(g + 1) * GS] = g * GS + slot
    invp = np.argsort(newpos)
    assert (batch[invp] == batch).all()
    dstp = newpos[dst]

    core = dstp // NPC
    local = dstp - core * NPC
    blk = local >> 7
    dl = local & 127

    seg = core * NBLK + blk
    n_seg = NC * NBLK
    order = np.lexsort((src, seg))
    seg_o = seg[order]
    seg_cnt = np.bincount(seg_o, minlength=n_seg)
    C = int(np.max(seg_cnt))
    C = max(128, -(-C // 128) * 128)
    cfg.C = C
    EP = cfg.EP

    seg_start = np.zeros(n_seg, np.int64)
    np.cumsum(seg_cnt[:-1], out=seg_start[1:])
    within = np.arange(len(order)) - seg_start[seg_o]
    pos = (seg_o % NBLK) * C + within          # core-relative padded pos
    core_o = seg_o // NBLK

    src_at = np.zeros((NC, EP), np.int64)
    src_at[core_o, pos] = src[order]
    dstl_at = np.full((NC, EP), 128.0, np.float32)
    dstl_at[core_o, pos] = dl[order].astype(np.float32)
    ea_at = np.zeros((NC, EP, cfg.FE), np.float32)
    ea_at[core_o, pos] = edge_attr[order]

    # eaT4: 4-phase layout. Edge position p (chunk c=p//128, lane e=p%128)
    # maps to [32*(c%4)+f, (c//4)*128+e] — each 128-col block is a shared
    # K=128 matmul lhsT covering 4 chunks (phase selection via the
    # block-diagonal We).
    G4 = EP // 512
    eaT4 = ea_at.reshape(NC, G4, 4, 128, cfg.FE).transpose(0, 2, 4, 1, 3) \
        .reshape(NC, 4 * cfg.FE, G4 * 128).astype(BF16)

    dstcol = dstl_at.reshape(NC, EP // 128, 128).transpose(0, 2, 1) \
        .astype(BF16).copy()

    # node-side arrays (new node order)
    xT = x[invp].reshape(NC, NPC, H).transpose(0, 2, 1) \
        .astype(np.float32).copy()

    f32 = lambda a: np.asarray(a, np.float32)
    xtab = (x + f32(be1)[None, :]).astype(F8)

    def bnfold(g, bt, m, v, b):
        A_ = f32(g) / np.sqrt(f32(v) + cfg.bn_eps)
        B_ = A_ * f32(b) + (f32(bt) - A_ * f32(m))
        return A_.reshape(-1, 1), B_.reshape(-1, 1)

    A1, B1 = bnfold(g1, bt1, m1, v1, b1)
    A2, B2 = bnfold(g2, bt2, m2, v2, b2)
    Aa1, Ba1 = bnfold(ga1, bta1, ma1, va1, ba1)
    Aa2, Ba2 = bnfold(ga2, bta2, ma2, va2, ba2)

    def wsel(We_):  # [128, 4*H]: block q has We at rows 32q..32q+31
        W_ = np.zeros((128, 4 * H), np.float32)
        for q in range(4):
            W_[32 * q:32 * q + cfg.FE, q * H:(q + 1) * H] = f32(We_)
        return W_.astype(BF16)

    wts = dict(
        We1=wsel(We1),
        We2=wsel(We2),
        W1=f32(W1).astype(BF16), W2=f32(W2).astype(BF16),
        A1=A1, B1=B1, A2=A2, B2=B2,
        be2=f32(be2),
        # mean pool (1/1024) folded into Wa1
        Wa1=f32(Wa1) / (cfg.N // cfg.NG), Aa1=Aa1, Ba1=Ba1,
        Wa2=f32(Wa2), Aa2=Aa2, Ba2=Ba2,
        Wa3=f32(Wa3), ba3=f32(ba3).reshape(-1, 1),
    )
    percore = dict(eaT4=eaT4, dstcol=dstcol, xT=xT, src_at=src_at,
                   newpos=newpos)
    return xtab, percore, wts


def pack_stream(tab, src_at, EP):
    """tab [N, 128] bf16, src_at [NC, EP] -> [NC, 128, EP] bf16 where
    out[c, lane, ch*128+f] = tab[src_at[c, ch*128+lane], f]."""
    NC = src_at.shape[0]
    g = tab[src_at.reshape(-1)]                    # [NC*EP, 128]
    g = g.reshape(NC, EP // 128, 128, 128)         # [c, ch, lane, f]
    return np.ascontiguousarray(g.transpose(0, 2, 1, 3)).reshape(NC, 128, EP)


# --------------------------------------------------------- bass programs ----

def build_program(cfg, launch):
    """launch: 1 (conv1 -> h) or 2 (conv2 + pool + head)."""
    import concourse.bacc as bacc
    import concourse.tile as tile
    from concourse import mybir
    from concourse.masks import make_identity

    dt = mybir.dt
    AF = mybir.ActivationFunctionType
    OP = mybir.AluOpType
    H = cfg.H
    NPC, NBLK, WBLK, NW = cfg.NPC, cfg.NBLK, cfg.WBLK, cfg.NW
    C, CPB, CPW, EPW, EP = cfg.C, cfg.CPB, cfg.CPW, cfg.EPW, cfg.EP
    assert CPW % 4 == 0
    NG4 = CPW // 4
    # batches of 1-2 four-chunk groups sharing one PSUM tile / ACT
    batches = [(2 * i, 2 * i + 1) for i in range(NG4 // 2)]
    if NG4 % 2:
        batches.append((NG4 - 1,))

    nc = bacc.Bacc("TRN2", target_bir_lowering=False, debug=False,
                   enable_asserts=False, num_devices=cfg.n_cores)

    din = lambda n, s, d: nc.dram_tensor(n, s, d, kind="ExternalInput").ap()
    dout = lambda n, s, d: nc.dram_tensor(n, s, d, kind="ExternalOutput").ap()

    xg = din("xg", [128, EP], dt.float8e4)
    eaT4 = din("eaT4", [128, EP // 4], dt.bfloat16)
    dstcol = din("dstcol", [128, EP // 128], dt.bfloat16)
    We = din("We", [128, 4 * H], dt.bfloat16)
    W = din("W", [H, H], dt.bfloat16)
    Asc = din("Asc", [H, 1], dt.float32)
    Bsc = din("Bsc", [H, 1], dt.float32)
    if launch == 1:
        xT = din("xT", [128, NPC], dt.float32)
        hT_out = dout("hT_out", [128, NPC], dt.bfloat16)
    else:
        xT = din("xT", [128, NPC], dt.bfloat16)
        Wa1 = din("Wa1", [H, H], dt.float32)
        Aa1 = din("Aa1", [H, 1], dt.float32)
        Ba1 = din("Ba1", [H, 1], dt.float32)
        Wa2 = din("Wa2", [H, H], dt.float32)
        Aa2 = din("Aa2", [H, 1], dt.float32)
        Ba2 = din("Ba2", [H, 1], dt.float32)
        Wa3 = din("Wa3", [H, cfg.A], dt.float32)
        ba3 = din("ba3", [cfg.A, 1], dt.float32)
        act_out = dout("act_out", [cfg.A, cfg.GPC], dt.float32)

    with tile.TileContext(nc) as tc:
        with (
            tc.tile_pool(name="const", bufs=1) as cpool,
            tc.tile_pool(name="xg", bufs=2) as xgpool,
            tc.tile_pool(name="stream", bufs=2) as spool,
            tc.tile_pool(name="sS", bufs=2) as spool_S,
            tc.tile_pool(name="work", bufs=3) as wpool,
            tc.tile_pool(name="blk", bufs=3) as bpool,
            tc.tile_pool(name="ps_t", bufs=2, space="PSUM") as ps_t,
            tc.tile_pool(name="ps_agg", bufs=2, space="PSUM") as ps_agg,
            tc.tile_pool(name="ps_misc", bufs=2, space="PSUM") as ps_misc,
        ):
            # ---- persistent constants
            dstcol_sb = cpool.tile([128, EP // 128], dt.bfloat16, tag="dstc")
            We_sb = cpool.tile([128, 4 * H], dt.bfloat16, tag="We")
            W_sb = cpool.tile([H, H], dt.bfloat16, tag="W")
            A_sb = cpool.tile([H, 1], dt.float32, tag="Asc")
            B_sb = cpool.tile([H, 1], dt.float32, tag="Bsc")
            nc.sync.dma_start(dstcol_sb[:], dstcol[:])
            nc.sync.dma_start(We_sb[:], We[:])
            nc.sync.dma_start(W_sb[:], W[:])
            nc.sync.dma_start(A_sb[:], Asc[:])
            nc.sync.dma_start(B_sb[:], Bsc[:])

            iota_sb = cpool.tile([128, 128], dt.bfloat16, tag="iota")
            nc.gpsimd.iota(iota_sb[:], pattern=[[1, 128]], base=0,
                           channel_multiplier=0,
                           allow_small_or_imprecise_dtypes=True)
            id_f8 = cpool.tile([128, 128], dt.float8e4, tag="idf8")
            make_identity(nc, id_f8[:])

            if launch == 2:
                Wa1_sb = cpool.tile([H, H], dt.float32, tag="Wa1")
                Wa2_sb = cpool.tile([H, H], dt.float32, tag="Wa2")
                Wa3_sb = cpool.tile([H, cfg.A], dt.float32, tag="Wa3")
                Aa1_sb = cpool.tile([H, 1], dt.float32, tag="Aa1")
                Ba1_sb = cpool.tile([H, 1], dt.float32, tag="Ba1")
                Aa2_sb = cpool.tile([H, 1], dt.float32, tag="Aa2")
                Ba2_sb = cpool.tile([H, 1], dt.float32, tag="Ba2")
                ba3_sb = cpool.tile([cfg.A, 1], dt.float32, tag="ba3")
                for a, b in ((Wa1_sb, Wa1), (Wa2_sb, Wa2), (Wa3_sb, Wa3),
                             (Aa1_sb, Aa1), (Ba1_sb, Ba1), (Aa2_sb, Aa2),
                             (Ba2_sb, Ba2), (ba3_sb, ba3)):
                    nc.sync.dma_start(a[:], b[:])
                bs_sb = cpool.tile([128, NBLK], dt.float32, tag="bs")

            # ---- main loop over windows (software-pipelined: each batch's
            # agg matmuls are emitted after the NEXT batch's t matmuls so
            # TensorE never head-of-line blocks on the ACT relu; each
            # window's drain is emitted after the next window's first batch)
            def emit_agg(msg, grp, S_list, agg_ps):
                for j in range(4 * len(grp)):
                    ch = grp[0] * 4 + j
                    bw, ci = divmod(ch, CPB)
                    nc.tensor.matmul(
                        agg_ps[:, bw * 128:(bw + 1) * 128],
                        lhsT=msg[:, j * 128:(j + 1) * 128],
                        rhs=S_list[bw][:, ci, :],
                        start=(ci == 0), stop=(ci == CPB - 1),
                        skip_group_check=True)

            def emit_drain(wdx, agg_ps, xt_sl):
                yT = wpool.tile([128, WBLK * 128], dt.bfloat16, tag="yT")
                nc.vector.tensor_tensor(out=yT[:], in0=agg_ps[:],
                                        in1=xt_sl[:], op=OP.add)
                for k in range(WBLK):
                    b_abs = wdx * WBLK + k
                    hp_ps = ps_misc.tile([128, 128], dt.float32, tag="m")
                    nc.tensor.matmul(hp_ps[:], lhsT=W_sb[:],
                                     rhs=yT[:, k * 128:(k + 1) * 128],
                                     start=True, stop=True,
                                     skip_group_check=True)
                    if launch == 1:
                        hT_t = bpool.tile([128, 128], dt.bfloat16, tag="hT")
                        nc.scalar.activation(hT_t[:], hp_ps[:], AF.Relu,
                                             bias=B_sb[:], scale=A_sb[:])
                        nc.sync.dma_start(
                            hT_out[:, b_abs * 128:(b_abs + 1) * 128], hT_t[:])
                    else:
                        # sigmoid(relu(z)) == max(sigmoid(z), 0.5)
                        sT = bpool.tile([128, 128], dt.float32, tag="sT")
                        nc.scalar.activation(sT[:], hp_ps[:], AF.Sigmoid,
                                             bias=B_sb[:], scale=A_sb[:])
                        h2T = bpool.tile([128, 128], dt.bfloat16, tag="h2T")
                        nc.vector.tensor_scalar(
                            out=h2T[:], in0=sT[:], scalar1=0.5, scalar2=0.0,
                            op0=OP.max, op1=OP.add,
                            accum_out=bs_sb[:, b_abs:b_abs + 1])

            pend_agg = None      # (msg, grp, S_list, agg_ps)
            pend_drain = None    # (wdx, agg_ps, xt_sl)
            for wdx in range(NW):
                xg_sl = xgpool.tile([128, EPW], dt.float8e4, tag="xg")
                nc.sync.dma_start(xg_sl[:],
                                  xg[:, wdx * EPW:(wdx + 1) * EPW])
                ea_sl = spool.tile([128, EPW // 4], dt.bfloat16, tag="ea")
                nc.sync.dma_start(
                    ea_sl[:], eaT4[:, wdx * (EPW // 4):(wdx + 1) * (EPW // 4)])
                xt_sl = spool.tile([128, WBLK * 128],
                                   dt.float32 if launch == 1 else dt.bfloat16,
                                   tag="xt", bufs=3)
                nc.sync.dma_start(xt_sl[:],
                                  xT[:, wdx * WBLK * 128:(wdx + 1) * WBLK * 128])

                # dst one-hot S per 128-dst block (CPB chunks each)
                S_blk = []
                for bw in range(WBLK):
                    c0 = wdx * CPW + bw * CPB
                    S_b = spool_S.tile([128, CPB, 128], dt.bfloat16,
                                       tag=f"S{bw}")
                    iota_b = iota_sb[:].unsqueeze(1) \
                        .to_broadcast([128, CPB, 128])
                    dst_b = dstcol_sb[:, c0:c0 + CPB].unsqueeze(2) \
                        .to_broadcast([128, CPB, 128])
                    if os.environ.get("BASS_GNN_GPS_S") and bw % 2 == 1:
                        nc.gpsimd.scalar_tensor_tensor(
                            out=S_b[:], in0=dst_b, scalar=0.0, in1=iota_b,
                            op0=OP.add, op1=OP.is_equal)
                    else:
                        nc.vector.tensor_tensor(
                            out=S_b[:], in0=iota_b, in1=dst_b,
                            op=OP.is_equal)
                    S_blk.append(S_b)

                agg_ps = ps_agg.tile([128, WBLK * 128], dt.float32, tag="agg")

                for grp in batches:
                    nw = 512 * len(grp)
                    t_ps = ps_t.tile([128, 1024], dt.float32, tag="t")
                    for gi, Gw in enumerate(grp):
                        lhs = ea_sl[:, Gw * 128:(Gw + 1) * 128]
                        nc.tensor.matmul(t_ps[:, gi * 512:(gi + 1) * 512],
                                         lhsT=lhs, rhs=We_sb[:],
                                         start=True, stop=False,
                                         skip_group_check=True)
                    for gi, Gw in enumerate(grp):
                        nc.tensor.matmul(t_ps[:, gi * 512:(gi + 1) * 512],
                                         lhsT=id_f8[:],
                                         rhs=xg_sl[:, Gw * 512:(Gw + 1) * 512],
                                         start=False, stop=True,
                                         skip_group_check=True)
                    msg = wpool.tile([128, 1024], dt.bfloat16, tag="msg")
                    nc.scalar.activation(msg[:, 0:nw], t_ps[:, 0:nw], AF.Relu)
                    if pend_agg is not None:
                        emit_agg(*pend_agg)
                    if pend_drain is not None:
                        emit_drain(*pend_drain)
                        pend_drain = None
                    pend_agg = (msg, grp, S_blk, agg_ps)
                pend_drain = (wdx, agg_ps, xt_sl)

            emit_agg(*pend_agg)
            emit_drain(*pend_drain)

            if launch == 2:
                # per-graph sums (graphs are 8 consecutive blocks), head
                pooledT = bpool.tile([128, cfg.GPC], dt.float32, tag="plT")
                for g in range(cfg.GPC):
                    nc.vector.tensor_reduce(
                        out=pooledT[:, g:g + 1],
                        in_=bs_sb[:, g * 8:(g + 1) * 8],
                        axis=mybir.AxisListType.X, op=OP.add)

                a1_ps = ps_misc.tile([128, cfg.GPC], dt.float32, tag="m")
                nc.tensor.matmul(a1_ps[:], lhsT=Wa1_sb[:], rhs=pooledT[:],
                                 start=True, stop=True, skip_group_check=True)
                a1 = bpool.tile([128, cfg.GPC], dt.float32, tag="a1")
                nc.scalar.activation(a1[:], a1_ps[:], AF.Relu,
                                     bias=Ba1_sb[:], scale=Aa1_sb[:])
                a2_ps = ps_misc.tile([128, cfg.GPC], dt.float32, tag="m")
                nc.tensor.matmul(a2_ps[:], lhsT=Wa2_sb[:], rhs=a1[:],
                                 start=True, stop=True, skip_group_check=True)
                a2 = bpool.tile([128, cfg.GPC], dt.float32, tag="a2")
                nc.scalar.activation(a2[:], a2_ps[:], AF.Relu,
                                     bias=Ba2_sb[:], scale=Aa2_sb[:])
                a3_ps = ps_misc.tile([cfg.A, cfg.GPC], dt.float32, tag="m")
                nc.tensor.matmul(a3_ps[:], lhsT=Wa3_sb[:], rhs=a2[:],
                                 start=True, stop=True, skip_group_check=True)
                a3 = bpool.tile([cfg.A, cfg.GPC], dt.float32, tag="a3")
                nc.scalar.activation(a3[:], a3_ps[:], AF.Sigmoid,
                                     bias=ba3_sb[:])
                nc.sync.dma_start(act_out[:], a3[:])

    nc.compile()
    return nc


# ------------------------------------------------------------- execution ----

def make_in_maps(cfg, launch, xg_pc, percore, wts, hT_percore=None):
    NC = cfg.n_cores
    maps = []
    for c in range(NC):
        m = dict(xg=np.ascontiguousarray(xg_pc[c]),
                 eaT4=np.ascontiguousarray(percore["eaT4"][c]),
                 dstcol=np.ascontiguousarray(percore["dstcol"][c]))
        if launch == 1:
            m.update(xT=np.ascontiguousarray(percore["xT"][c]),
                     We=wts["We1"], W=wts["W1"], Asc=wts["A1"], Bsc=wts["B1"])
        else:
            m.update(xT=np.ascontiguousarray(hT_percore[c]),
                     We=wts["We2"], W=wts["W2"], Asc=wts["A2"], Bsc=wts["B2"],
                     Wa1=wts["Wa1"], Aa1=wts["Aa1"], Ba1=wts["Ba1"],
                     Wa2=wts["Wa2"], Aa2=wts["Aa2"], Ba2=wts["Ba2"],
                     Wa3=wts["Wa3"], ba3=wts["ba3"])
        maps.append(m)
    return maps


_PROG_CACHE = {}
LAST_EXEC_NS = {}


def kernel(**inputs):
    from concourse import bass_utils
    _enable_ldw_opt()

    cfg = Cfg()
    xtab, percore, wts = host_prep(cfg, **inputs)

    key = (cfg.N, cfg.E, cfg.C)
    if key not in _PROG_CACHE:
        _PROG_CACHE[key] = (build_program(cfg, 1), build_program(cfg, 2))
    nc1, nc2 = _PROG_CACHE[key]

    trace = bool(int(os.environ.get("BASS_GNN_TRACE", "0")))
    core_ids = list(range(cfg.n_cores))

    xg1 = pack_stream(xtab, percore["src_at"], cfg.EP)
    maps1 = make_in_maps(cfg, 1, xg1, percore, wts)
    res1 = bass_utils.run_bass_kernel_spmd(nc1, maps1, core_ids=core_ids,
                                           trace=trace)
    LAST_EXEC_NS["L1"] = res1.exec_time_ns
    if os.environ.get("BASS_GNN_ONLY_L1"):
        return res1
    hT = [res1.results[c]["hT_out"] for c in core_ids]      # [128, NPC] bf16

    h_all = np.concatenate([t.T for t in hT], axis=0)       # [N, H] new order
    h_orig = h_all[percore["newpos"]]                       # rows by orig id
    htab = (h_orig.astype(np.float32) + wts["be2"][None, :]).astype(F8)
    xg2 = pack_stream(htab, percore["src_at"], cfg.EP)

    maps2 = make_in_maps(cfg, 2, xg2, percore, wts, hT_percore=hT)
    res2 = bass_utils.run_bass_kernel_spmd(nc2, maps2, core_ids=core_ids,
                                           trace=trace)
    LAST_EXEC_NS["L2"] = res2.exec_time_ns

    out = np.zeros((cfg.NG, cfg.A), np.float32)
    for c in core_ids:
        a3 = res2.results[c]["act_out"]          # [A, GPC]
        out[c * cfg.GPC:(c + 1) * cfg.GPC, :] = a3.T
    return out


# revision 42
# speedup vs baseline: 1.0872x; 1.0341x over previous
"""Trainium2 Bass kernel for nn_ActionModel (2x GINEConv + mean-pool + MLP head).

Strategy (8 NeuronCores, SPMD):
  - Nodes sharded by graph: core m owns 8 consecutive graphs = 8192 nodes.
  - Edges sharded by dst owner; per core, edges are grouped by 128-dst block,
    padded to a fixed per-block capacity C so the instruction stream is
    identical across cores.
  - Host prep builds, per core, sequentially-streamable operand arrays in
    padded edge order (the same treatment the edge_attr already gets):
      * xg  : x[src]+be (bf16) laid out [128 lanes, chunk, feat]
      * eaT4: edge_attr 4-phase packed so one K=128 matmul against a
              block-diagonal We computes ea@We for 4 chunks at once
      * dstcol: per-edge dst-local-in-block (bf16, 128 = padding sentinel)
  - On-device, per 1024-edge pair of 4-chunk groups:
      TensorE: ea@We (one N=512 matmul per group) + identity-matmul add of
      xg into PSUM; ACT applies ReLU over [128,1024] -> bf16 msg; DVE builds
      the dst one-hot S per 128-dst block (iota/is_equal); TensorE
      accumulates aggT += msg^T @ S into [feat, dst] PSUM.
  - Node stage: yT = aggT + xT; Linear+folded-BN+ReLU via TensorE/ACT.
  - Two launches: L1 -> hT (bf16); host rebuilds the conv2 edge stream
    (h+be2)[src]; L2 runs conv2, sigmoid with per-block accum_out giving
    block sums, per-graph mean pool (graphs are contiguous 1024-node
    ranges), and the 3-layer head. Only [A, GPC] per core comes back.
"""

import heapq
import os
import sys
import numpy as np

for _p in ("/opt/trn_rl_repo",):
    if _p not in sys.path and os.path.isdir(_p):
        sys.path.insert(0, _p)

import ml_dtypes  # noqa: E402

BF16 = ml_dtypes.bfloat16
F8 = ml_dtypes.float8_e4m3


def _enable_ldw_opt():
    """Flip walrus's --enable-ldw-opt to true (merges/accelerates redundant
    LDWEIGHTS). Wraps concourse.bass_utils.run_command."""
    # walrus rejects bass-emitted InstLdweights under ldw-opt; keep off
    # unless explicitly requested for experiments.
    if not os.environ.get("BASS_GNN_LDWOPT"):
        return
    from concourse import bass_utils as _bu
    if getattr(_bu, "_gnn_ldwopt_patched", False):
        return
    _orig = _bu.run_command

    def _patched(cmd, *a, **k):
        if isinstance(cmd, list):
            cmd = ["--enable-ldw-opt=true" if c == "--enable-ldw-opt=false"
                   else c for c in cmd]
        return _orig(cmd, *a, **k)

    _bu.run_command = _patched
    _bu._gnn_ldwopt_patched = True

# ---------------------------------------------------------------- config ----

class Cfg:
    def __init__(self, N=65536, E=1048576, H=128, FE=32, NG=64, A=32,
                 n_cores=8, WBLK=4, bn_eps=1e-5):
        self.N, self.E, self.H, self.FE, self.NG, self.A = N, E, H, FE, NG, A
        self.n_cores = n_cores
        self.WBLK = WBLK          # dst blocks per window
        self.bn_eps = bn_eps
        self.NPC = N // n_cores   # nodes per core
        self.GPC = NG // n_cores  # graphs per core
        self.NBLK = self.NPC // 128
        assert self.NPC % 128 == 0 and self.NBLK % WBLK == 0
        self.NW = self.NBLK // WBLK
        self.C = None             # per-block capacity; set by prep

    @property
    def CPB(self):  # chunks per block
        return self.C // 128

    @property
    def CPW(self):  # chunks per window
        return self.WBLK * self.CPB

    @property
    def EPW(self):  # padded edge positions per window
        return self.CPW * 128

    @property
    def EP(self):   # padded edge positions per core
        return self.NBLK * self.C


# ------------------------------------------------------------- host prep ----

def host_prep(cfg, x, edge_index, edge_attr, batch,
              We1, be1, W1, b1, g1, bt1, m1, v1,
              We2, be2, W2, b2, g2, bt2, m2, v2,
              Wa1, ba1, ga1, bta1, ma1, va1,
              Wa2, ba2, ga2, bta2, ma2, va2,
              Wa3, ba3):
    """Partition/sort/pad edges, build per-core streamable arrays."""
    N, H, NC = cfg.N, cfg.H, cfg.n_cores
    NPC, NBLK = cfg.NPC, cfg.NBLK

    src = np.asarray(edge_index[0], dtype=np.int64)
    dst = np.asarray(edge_index[1], dtype=np.int64)
    batch = np.asarray(batch, dtype=np.int64)
    x = np.asarray(x, dtype=np.float32)
    edge_attr = np.asarray(edge_attr, dtype=np.float32)

    cnts = np.bincount(batch, minlength=cfg.NG)
    assert (cnts == cfg.N // cfg.NG).all(), "equal-size graphs expected"

    # Within-graph node relabeling balancing per-block in-degree (greedy
    # first-fit-decreasing into the 8 blocks of each graph). Shrinks the
    # padded per-block capacity C. Pooling is within-graph permutation
    # invariant; the gather table stays in original node ids.
    GS = N // cfg.NG
    BPG = GS // 128
    indeg = np.bincount(dst, minlength=N)
    newpos = np.empty(N, np.int64)
    for g in range(cfg.NG):
        deg = indeg[g * GS:(g + 1) * GS]
        order_g = np.argsort(-deg, kind="stable")
        heap = [(0, 0, b) for b in range(BPG)]
        heapq.heapify(heap)
        slot = np.empty(GS, np.int64)
        for nd in order_g:
            load, c, b = heapq.heappop(heap)
            slot[nd] = b * 128 + c
            load += int(deg[nd])
            c += 1
            if c < 128:
                heapq.heappush(heap, (load, c, b))
        newpos[g * GS:(g + 1) * GS] = g * GS + slot
    invp = np.argsort(newpos)
    assert (batch[invp] == batch).all()
    dstp = newpos[dst]

    core = dstp // NPC
    local = dstp - core * NPC
    blk = local >> 7
    dl = local & 127

    seg = core * NBLK + blk
    n_seg = NC * NBLK
    order = np.lexsort((src, seg))
    seg_o = seg[order]
    seg_cnt = np.bincount(seg_o, minlength=n_seg)
    C = int(np.max(seg_cnt))
    C = max(128, -(-C // 128) * 128)
    cfg.C = C
    EP = cfg.EP

    seg_start = np.zeros(n_seg, np.int64)
    np.cumsum(seg_cnt[:-1], out=seg_start[1:])
    within = np.arange(len(order)) - seg_start[seg_o]
    pos = (seg_o % NBLK) * C + within          # core-relative padded pos
    core_o = seg_o // NBLK

    src_at = np.zeros((NC, EP), np.int64)
    src_at[core_o, pos] = src[order]
    dstl_at = np.full((NC, EP), 128.0, np.float32)
    dstl_at[core_o, pos] = dl[order].astype(np.float32)
    ea_at = np.zeros((NC, EP, cfg.FE), np.float32)
    ea_at[core_o, pos] = edge_attr[order]

    # eaT4: 4-phase layout. Edge position p (chunk c=p//128, lane e=p%128)
    # maps to [32*(c%4)+f, (c//4)*128+e] — each 128-col block is a shared
    # K=128 matmul lhsT covering 4 chunks (phase selection via the
    # block-diagonal We).
    G4 = EP // 512
    eaT4 = ea_at.reshape(NC, G4, 4, 128, cfg.FE).transpose(0, 2, 4, 1, 3) \
        .reshape(NC, 4 * cfg.FE, G4 * 128).astype(BF16)

    dstcol = dstl_at.reshape(NC, EP // 128, 128).transpose(0, 2, 1) \
        .astype(BF16).copy()

    # node-side arrays (new node order)
    xT = x[invp].reshape(NC, NPC, H).transpose(0, 2, 1) \
        .astype(np.float32).copy()

    f32 = lambda a: np.asarray(a, np.float32)
    xtab = (x + f32(be1)[None, :]).astype(F8)

    def bnfold(g, bt, m, v, b):
        A_ = f32(g) / np.sqrt(f32(v) + cfg.bn_eps)
        B_ = A_ * f32(b) + (f32(bt) - A_ * f32(m))
        return A_.reshape(-1, 1), B_.reshape(-1, 1)

    A1, B1 = bnfold(g1, bt1, m1, v1, b1)
    A2, B2 = bnfold(g2, bt2, m2, v2, b2)
    Aa1, Ba1 = bnfold(ga1, bta1, ma1, va1, ba1)
    Aa2, Ba2 = bnfold(ga2, bta2, ma2, va2, ba2)

    def wsel(We_):  # [128, 4*H]: block q has We at rows 32q..32q+31
        W_ = np.zeros((128, 4 * H), np.float32)
        for q in range(4):
            W_[32 * q:32 * q + cfg.FE, q * H:(q + 1) * H] = f32(We_)
        return W_.astype(BF16)

    wts = dict(
        We1=wsel(We1),
        We2=wsel(We2),
        W1=f32(W1).astype(BF16), W2=f32(W2).astype(BF16),
        A1=A1, B1=B1, A2=A2, B2=B2,
        be2=f32(be2),
        # mean pool (1/1024) folded into Wa1
        Wa1=f32(Wa1) / (cfg.N // cfg.NG), Aa1=Aa1, Ba1=Ba1,
        Wa2=f32(Wa2), Aa2=Aa2, Ba2=Ba2,
        Wa3=f32(Wa3), ba3=f32(ba3).reshape(-1, 1),
    )
    percore = dict(eaT4=eaT4, dstcol=dstcol, xT=xT, src_at=src_at,
                   newpos=newpos)
    return xtab, percore, wts


def pack_stream(tab, src_at, EP):
    """tab [N, 128] bf16, src_at [NC, EP] -> [NC, 128, EP] bf16 where
    out[c, lane, ch*128+f] = tab[src_at[c, ch*128+lane], f]."""
    NC = src_at.shape[0]
    g = tab[src_at.reshape(-1)]                    # [NC*EP, 128]
    g = g.reshape(NC, EP // 128, 128, 128)         # [c, ch, lane, f]
    return np.ascontiguousarray(g.transpose(0, 2, 1, 3)).reshape(NC, 128, EP)


# --------------------------------------------------------- bass programs ----

def build_program(cfg, launch):
    """launch: 1 (conv1 -> h) or 2 (conv2 + pool + head)."""
    import concourse.bacc as bacc
    import concourse.tile as tile
    from concourse import mybir
    from concourse.masks import make_identity

    dt = mybir.dt
    AF = mybir.ActivationFunctionType
    OP = mybir.AluOpType
    H = cfg.H
    NPC, NBLK, WBLK, NW = cfg.NPC, cfg.NBLK, cfg.WBLK, cfg.NW
    C, CPB, CPW, EPW, EP = cfg.C, cfg.CPB, cfg.CPW, cfg.EPW, cfg.EP
    assert CPW % 4 == 0
    NG4 = CPW // 4
    # batches of 1-2 four-chunk groups sharing one PSUM tile / ACT
    batches = [(2 * i, 2 * i + 1) for i in range(NG4 // 2)]
    if NG4 % 2:
        batches.append((NG4 - 1,))

    nc = bacc.Bacc("TRN2", target_bir_lowering=False, debug=False,
                   enable_asserts=False, num_devices=cfg.n_cores)

    din = lambda n, s, d: nc.dram_tensor(n, s, d, kind="ExternalInput").ap()
    dout = lambda n, s, d: nc.dram_tensor(n, s, d, kind="ExternalOutput").ap()

    EPC = EP // 128
    CB16 = 4 * H + H + EPC           # We | W | dstcol
    CF32 = 2 if launch == 1 else 2 + H + 2 + H + 2 + cfg.A + 1
    xg = din("xg", [128, EP], dt.float8e4)
    eaT4 = din("eaT4", [128, EP // 4], dt.bfloat16)
    cb16 = din("cb16", [128, CB16], dt.bfloat16)
    cf32 = din("cf32", [128, CF32], dt.float32)
    if launch == 1:
        xT = din("xT", [128, NPC], dt.float32)
        hT_out = dout("hT_out", [128, NPC], dt.bfloat16)
    else:
        xT = din("xT", [128, NPC], dt.bfloat16)
        act_out = dout("act_out", [cfg.A, cfg.GPC], dt.float32)

    with tile.TileContext(nc) as tc:
        with (
            tc.tile_pool(name="const", bufs=1) as cpool,
            tc.tile_pool(name="xg", bufs=2) as xgpool,
            tc.tile_pool(name="stream", bufs=2) as spool,
            tc.tile_pool(name="sS", bufs=2) as spool_S,
            tc.tile_pool(name="work", bufs=3) as wpool,
            tc.tile_pool(name="blk", bufs=3) as bpool,
            tc.tile_pool(name="ps_t", bufs=2, space="PSUM") as ps_t,
            tc.tile_pool(name="ps_agg", bufs=2, space="PSUM") as ps_agg,
            tc.tile_pool(name="ps_misc", bufs=2, space="PSUM") as ps_misc,
        ):
            # ---- persistent constants: two blob DMAs, views by column slice
            cb16_sb = cpool.tile([128, CB16], dt.bfloat16, tag="cb16")
            cf32_sb = cpool.tile([128, CF32], dt.float32, tag="cf32")
            nc.sync.dma_start(cb16_sb[:], cb16[:])
            nc.sync.dma_start(cf32_sb[:], cf32[:])
            We_sb = cb16_sb[:, 0:4 * H]
            W_sb = cb16_sb[:, 4 * H:5 * H]
            dstcol_sb = cb16_sb[:, 5 * H:5 * H + EPC]
            A_sb = cf32_sb[:, 0:1]
            B_sb = cf32_sb[:, 1:2]

            iota_sb = cpool.tile([128, 128], dt.bfloat16, tag="iota")
            nc.gpsimd.iota(iota_sb[:], pattern=[[1, 128]], base=0,
                           channel_multiplier=0,
                           allow_small_or_imprecise_dtypes=True)
            id_f8 = cpool.tile([128, 128], dt.float8e4, tag="idf8")
            make_identity(nc, id_f8[:])

            if launch == 2:
                o = 2
                Wa1_sb = cf32_sb[:, o:o + H]; o += H
                Aa1_sb = cf32_sb[:, o:o + 1]; o += 1
                Ba1_sb = cf32_sb[:, o:o + 1]; o += 1
                Wa2_sb = cf32_sb[:, o:o + H]; o += H
                Aa2_sb = cf32_sb[:, o:o + 1]; o += 1
                Ba2_sb = cf32_sb[:, o:o + 1]; o += 1
                Wa3_sb = cf32_sb[:, o:o + cfg.A]; o += cfg.A
                ba3_sb = cf32_sb[0:cfg.A, o:o + 1]; o += 1
                bs_sb = cpool.tile([128, NBLK], dt.float32, tag="bs")

            # ---- main loop over windows (software-pipelined: each batch's
            # agg matmuls are emitted after the NEXT batch's t matmuls so
            # TensorE never head-of-line blocks on the ACT relu; each
            # window's drain is emitted after the next window's first batch)
            def emit_agg(msg, grp, S_list, agg_ps):
                for j in range(4 * len(grp)):
                    ch = grp[0] * 4 + j
                    bw, ci = divmod(ch, CPB)
                    nc.tensor.matmul(
                        agg_ps[:, bw * 128:(bw + 1) * 128],
                        lhsT=msg[:, j * 128:(j + 1) * 128],
                        rhs=S_list[bw][:, ci, :],
                        start=(ci == 0), stop=(ci == CPB - 1),
                        skip_group_check=True)

            def emit_drain(wdx, agg_ps, xt_sl):
                yT = wpool.tile([128, WBLK * 128], dt.bfloat16, tag="yT")
                nc.vector.tensor_tensor(out=yT[:], in0=agg_ps[:],
                                        in1=xt_sl[:], op=OP.add)
                if launch == 1:
                    hTw = bpool.tile([128, WBLK * 128], dt.bfloat16,
                                     tag="hTw", bufs=2)
                for k in range(WBLK):
                    b_abs = wdx * WBLK + k
                    hp_ps = ps_misc.tile([128, 128], dt.float32, tag="m")
                    nc.tensor.matmul(hp_ps[:], lhsT=W_sb,
                                     rhs=yT[:, k * 128:(k + 1) * 128],
                                     start=True, stop=True,
                                     skip_group_check=True)
                    if launch == 1:
                        nc.scalar.activation(hTw[:, k * 128:(k + 1) * 128],
                                             hp_ps[:], AF.Relu,
                                             bias=B_sb, scale=A_sb)
                    else:
                        # sigmoid(relu(z)) == max(sigmoid(z), 0.5)
                        sT = bpool.tile([128, 128], dt.float32, tag="sT")
                        nc.scalar.activation(sT[:], hp_ps[:], AF.Sigmoid,
                                             bias=B_sb, scale=A_sb)
                        h2T = bpool.tile([128, 128], dt.bfloat16, tag="h2T")
                        nc.vector.tensor_scalar(
                            out=h2T[:], in0=sT[:], scalar1=0.5, scalar2=0.0,
                            op0=OP.max, op1=OP.add,
                            accum_out=bs_sb[:, b_abs:b_abs + 1])
                if launch == 1:
                    nc.sync.dma_start(
                        hT_out[:, wdx * WBLK * 128:(wdx + 1) * WBLK * 128],
                        hTw[:])

            pend_agg = None      # (msg, grp, S_list, agg_ps)
            pend_drain = []      # [slots_left, (wdx, agg_ps, xt_sl)]
            for wdx in range(NW):
                xg_sl = xgpool.tile([128, EPW], dt.float8e4, tag="xg")
                nc.sync.dma_start(xg_sl[:],
                                  xg[:, wdx * EPW:(wdx + 1) * EPW])
                ea_sl = spool.tile([128, EPW // 4], dt.bfloat16, tag="ea")
                nc.sync.dma_start(
                    ea_sl[:], eaT4[:, wdx * (EPW // 4):(wdx + 1) * (EPW // 4)])
                xt_sl = spool.tile([128, WBLK * 128],
                                   dt.float32 if launch == 1 else dt.bfloat16,
                                   tag="xt", bufs=3)
                nc.sync.dma_start(xt_sl[:],
                                  xT[:, wdx * WBLK * 128:(wdx + 1) * WBLK * 128])

                # dst one-hot S per 128-dst block (CPB chunks each)
                S_blk = []
                for bw in range(WBLK):
                    c0 = wdx * CPW + bw * CPB
                    S_b = spool_S.tile([128, CPB, 128], dt.bfloat16,
                                       tag=f"S{bw}")
                    iota_b = iota_sb[:].unsqueeze(1) \
                        .to_broadcast([128, CPB, 128])
                    dst_b = dstcol_sb[:, c0:c0 + CPB].unsqueeze(2) \
                        .to_broadcast([128, CPB, 128])
                    if os.environ.get("BASS_GNN_GPS_S") and bw % 2 == 1:
                        nc.gpsimd.scalar_tensor_tensor(
                            out=S_b[:], in0=dst_b, scalar=0.0, in1=iota_b,
                            op0=OP.add, op1=OP.is_equal)
                    else:
                        nc.vector.tensor_tensor(
                            out=S_b[:], in0=iota_b, in1=dst_b,
                            op=OP.is_equal)
                    S_blk.append(S_b)

                agg_ps = ps_agg.tile([128, WBLK * 128], dt.float32, tag="agg")

                for grp in batches:
                    nw = 512 * len(grp)
                    t_ps = ps_t.tile([128, 1024], dt.float32, tag="t")
                    for gi, Gw in enumerate(grp):
                        lhs = ea_sl[:, Gw * 128:(Gw + 1) * 128]
                        nc.tensor.matmul(t_ps[:, gi * 512:(gi + 1) * 512],
                                         lhsT=lhs, rhs=We_sb,
                                         start=True, stop=False,
                                         skip_group_check=True)
                    for gi, Gw in enumerate(grp):
                        nc.tensor.matmul(t_ps[:, gi * 512:(gi + 1) * 512],
                                         lhsT=id_f8[:],
                                         rhs=xg_sl[:, Gw * 512:(Gw + 1) * 512],
                                         start=False, stop=True,
                                         skip_group_check=True)
                    msg = wpool.tile([128, 1024], dt.bfloat16, tag="msg")
                    nc.scalar.activation(msg[:, 0:nw], t_ps[:, 0:nw], AF.Relu)
                    if pend_agg is not None:
                        emit_agg(*pend_agg)
                    for d in pend_drain:
                        d[0] -= 1
                    if pend_drain and pend_drain[0][0] <= 0:
                        emit_drain(*pend_drain.pop(0)[1])
                    pend_agg = (msg, grp, S_blk, agg_ps)
                pend_drain.append([2, (wdx, agg_ps, xt_sl)])

            emit_agg(*pend_agg)
            for _, args in pend_drain:
                emit_drain(*args)

            if launch == 2:
                # per-graph sums (graphs are 8 consecutive blocks), head
                pooledT = bpool.tile([128, cfg.GPC], dt.float32, tag="plT")
                for g in range(cfg.GPC):
                    nc.vector.tensor_reduce(
                        out=pooledT[:, g:g + 1],
                        in_=bs_sb[:, g * 8:(g + 1) * 8],
                        axis=mybir.AxisListType.X, op=OP.add)

                a1_ps = ps_misc.tile([128, cfg.GPC], dt.float32, tag="m")
                nc.tensor.matmul(a1_ps[:], lhsT=Wa1_sb, rhs=pooledT[:],
                                 start=True, stop=True, skip_group_check=True)
                a1 = bpool.tile([128, cfg.GPC], dt.float32, tag="a1")
                nc.scalar.activation(a1[:], a1_ps[:], AF.Relu,
                                     bias=Ba1_sb, scale=Aa1_sb)
                a2_ps = ps_misc.tile([128, cfg.GPC], dt.float32, tag="m")
                nc.tensor.matmul(a2_ps[:], lhsT=Wa2_sb, rhs=a1[:],
                                 start=True, stop=True, skip_group_check=True)
                a2 = bpool.tile([128, cfg.GPC], dt.float32, tag="a2")
                nc.scalar.activation(a2[:], a2_ps[:], AF.Relu,
                                     bias=Ba2_sb, scale=Aa2_sb)
                a3_ps = ps_misc.tile([cfg.A, cfg.GPC], dt.float32, tag="m")
                nc.tensor.matmul(a3_ps[:], lhsT=Wa3_sb, rhs=a2[:],
                                 start=True, stop=True, skip_group_check=True)
                a3 = bpool.tile([cfg.A, cfg.GPC], dt.float32, tag="a3")
                nc.scalar.activation(a3[:], a3_ps[:], AF.Sigmoid,
                                     bias=ba3_sb)
                nc.sync.dma_start(act_out[:], a3[:])

    nc.compile()
    return nc


# ------------------------------------------------------------- execution ----

def make_in_maps(cfg, launch, xg_pc, percore, wts, hT_percore=None):
    NC = cfg.n_cores
    f32 = np.float32
    if launch == 1:
        cf32_shared = np.concatenate([wts["A1"], wts["B1"]], axis=1) \
            .astype(f32)
        We_, W_ = wts["We1"], wts["W1"]
    else:
        ba3p = np.zeros((128, 1), f32)
        ba3p[:cfg.A] = wts["ba3"]
        cf32_shared = np.concatenate(
            [wts["A2"], wts["B2"], wts["Wa1"], wts["Aa1"], wts["Ba1"],
             wts["Wa2"], wts["Aa2"], wts["Ba2"], wts["Wa3"], ba3p],
            axis=1).astype(f32)
        We_, W_ = wts["We2"], wts["W2"]
    maps = []
    for c in range(NC):
        cb16 = np.concatenate(
            [We_, W_, percore["dstcol"][c]], axis=1).astype(BF16)
        m = dict(xg=np.ascontiguousarray(xg_pc[c]),
                 eaT4=np.ascontiguousarray(percore["eaT4"][c]),
                 cb16=np.ascontiguousarray(cb16),
                 cf32=np.ascontiguousarray(cf32_shared))
        if launch == 1:
            m.update(xT=np.ascontiguousarray(percore["xT"][c]))
        else:
            m.update(xT=np.ascontiguousarray(hT_percore[c]))
        maps.append(m)
    return maps


_PROG_CACHE = {}
LAST_EXEC_NS = {}


def kernel(**inputs):
    from concourse import bass_utils
    _enable_ldw_opt()

    cfg = Cfg()
    xtab, percore, wts = host_prep(cfg, **inputs)

    key = (cfg.N, cfg.E, cfg.C)
    if key not in _PROG_CACHE:
        _PROG_CACHE[key] = (build_program(cfg, 1), build_program(cfg, 2))
    nc1, nc2 = _PROG_CACHE[key]

    trace = bool(int(os.environ.get("BASS_GNN_TRACE", "0")))
    core_ids = list(range(cfg.n_cores))

    xg1 = pack_stream(xtab, percore["src_at"], cfg.EP)
    maps1 = make_in_maps(cfg, 1, xg1, percore, wts)
    res1 = bass_utils.run_bass_kernel_spmd(nc1, maps1, core_ids=core_ids,
                                           trace=trace)
    LAST_EXEC_NS["L1"] = res1.exec_time_ns
    if os.environ.get("BASS_GNN_ONLY_L1"):
        return res1
    hT = [res1.results[c]["hT_out"] for c in core_ids]      # [128, NPC] bf16

    h_all = np.concatenate([t.T for t in hT], axis=0)       # [N, H] new order
    h_orig = h_all[percore["newpos"]]                       # rows by orig id
    htab = (h_orig.astype(np.float32) + wts["be2"][None, :]).astype(F8)
    xg2 = pack_stream(htab, percore["src_at"], cfg.EP)

    maps2 = make_in_maps(cfg, 2, xg2, percore, wts, hT_percore=hT)
    res2 = bass_utils.run_bass_kernel_spmd(nc2, maps2, core_ids=core_ids,
                                           trace=trace)
    LAST_EXEC_NS["L2"] = res2.exec_time_ns

    out = np.zeros((cfg.NG, cfg.A), np.float32)
    for c in core_ids:
        a3 = res2.results[c]["act_out"]          # [A, GPC]
        out[c * cfg.GPC:(c + 1) * cfg.GPC, :] = a3.T
    return out


# revision 43
# speedup vs baseline: 1.0886x; 1.0013x over previous
"""Trainium2 Bass kernel for nn_ActionModel (2x GINEConv + mean-pool + MLP head).

Strategy (8 NeuronCores, SPMD):
  - Nodes sharded by graph: core m owns 8 consecutive graphs = 8192 nodes.
  - Edges sharded by dst owner; per core, edges are grouped by 128-dst block,
    padded to a fixed per-block capacity C so the instruction stream is
    identical across cores.
  - Host prep builds, per core, sequentially-streamable operand arrays in
    padded edge order (the same treatment the edge_attr already gets):
      * xg  : x[src]+be (bf16) laid out [128 lanes, chunk, feat]
      * eaT4: edge_attr 4-phase packed so one K=128 matmul against a
              block-diagonal We computes ea@We for 4 chunks at once
      * dstcol: per-edge dst-local-in-block (bf16, 128 = padding sentinel)
  - On-device, per 1024-edge pair of 4-chunk groups:
      TensorE: ea@We (one N=512 matmul per group) + identity-matmul add of
      xg into PSUM; ACT applies ReLU over [128,1024] -> bf16 msg; DVE builds
      the dst one-hot S per 128-dst block (iota/is_equal); TensorE
      accumulates aggT += msg^T @ S into [feat, dst] PSUM.
  - Node stage: yT = aggT + xT; Linear+folded-BN+ReLU via TensorE/ACT.
  - Two launches: L1 -> hT (bf16); host rebuilds the conv2 edge stream
    (h+be2)[src]; L2 runs conv2, sigmoid with per-block accum_out giving
    block sums, per-graph mean pool (graphs are contiguous 1024-node
    ranges), and the 3-layer head. Only [A, GPC] per core comes back.
"""

import heapq
import os
import sys
import numpy as np

for _p in ("/opt/trn_rl_repo",):
    if _p not in sys.path and os.path.isdir(_p):
        sys.path.insert(0, _p)

import ml_dtypes  # noqa: E402

BF16 = ml_dtypes.bfloat16
F8 = ml_dtypes.float8_e4m3


def _enable_ldw_opt():
    """Flip walrus's --enable-ldw-opt to true (merges/accelerates redundant
    LDWEIGHTS). Wraps concourse.bass_utils.run_command."""
    # walrus rejects bass-emitted InstLdweights under ldw-opt; keep off
    # unless explicitly requested for experiments.
    if not os.environ.get("BASS_GNN_LDWOPT"):
        return
    from concourse import bass_utils as _bu
    if getattr(_bu, "_gnn_ldwopt_patched", False):
        return
    _orig = _bu.run_command

    def _patched(cmd, *a, **k):
        if isinstance(cmd, list):
            cmd = ["--enable-ldw-opt=true" if c == "--enable-ldw-opt=false"
                   else c for c in cmd]
        return _orig(cmd, *a, **k)

    _bu.run_command = _patched
    _bu._gnn_ldwopt_patched = True

# ---------------------------------------------------------------- config ----

class Cfg:
    def __init__(self, N=65536, E=1048576, H=128, FE=32, NG=64, A=32,
                 n_cores=8, WBLK=4, bn_eps=1e-5):
        self.N, self.E, self.H, self.FE, self.NG, self.A = N, E, H, FE, NG, A
        self.n_cores = n_cores
        self.WBLK = WBLK          # dst blocks per window
        self.bn_eps = bn_eps
        self.NPC = N // n_cores   # nodes per core
        self.GPC = NG // n_cores  # graphs per core
        self.NBLK = self.NPC // 128
        assert self.NPC % 128 == 0 and self.NBLK % WBLK == 0
        self.NW = self.NBLK // WBLK
        self.C = None             # per-block capacity; set by prep

    @property
    def CPB(self):  # chunks per block
        return self.C // 128

    @property
    def CPW(self):  # chunks per window
        return self.WBLK * self.CPB

    @property
    def EPW(self):  # padded edge positions per window
        return self.CPW * 128

    @property
    def EP(self):   # padded edge positions per core
        return self.NBLK * self.C


# ------------------------------------------------------------- host prep ----

def host_prep(cfg, x, edge_index, edge_attr, batch,
              We1, be1, W1, b1, g1, bt1, m1, v1,
              We2, be2, W2, b2, g2, bt2, m2, v2,
              Wa1, ba1, ga1, bta1, ma1, va1,
              Wa2, ba2, ga2, bta2, ma2, va2,
              Wa3, ba3):
    """Partition/sort/pad edges, build per-core streamable arrays."""
    N, H, NC = cfg.N, cfg.H, cfg.n_cores
    NPC, NBLK = cfg.NPC, cfg.NBLK

    src = np.asarray(edge_index[0], dtype=np.int64)
    dst = np.asarray(edge_index[1], dtype=np.int64)
    batch = np.asarray(batch, dtype=np.int64)
    x = np.asarray(x, dtype=np.float32)
    edge_attr = np.asarray(edge_attr, dtype=np.float32)

    cnts = np.bincount(batch, minlength=cfg.NG)
    assert (cnts == cfg.N // cfg.NG).all(), "equal-size graphs expected"

    # Within-graph node relabeling balancing per-block in-degree (greedy
    # first-fit-decreasing into the 8 blocks of each graph). Shrinks the
    # padded per-block capacity C. Pooling is within-graph permutation
    # invariant; the gather table stays in original node ids.
    GS = N // cfg.NG
    BPG = GS // 128
    indeg = np.bincount(dst, minlength=N)
    newpos = np.empty(N, np.int64)
    for g in range(cfg.NG):
        deg = indeg[g * GS:(g + 1) * GS]
        order_g = np.argsort(-deg, kind="stable")
        heap = [(0, 0, b) for b in range(BPG)]
        heapq.heapify(heap)
        slot = np.empty(GS, np.int64)
        for nd in order_g:
            load, c, b = heapq.heappop(heap)
            slot[nd] = b * 128 + c
            load += int(deg[nd])
            c += 1
            if c < 128:
                heapq.heappush(heap, (load, c, b))
        newpos[g * GS:(g + 1) * GS] = g * GS + slot
    invp = np.argsort(newpos)
    assert (batch[invp] == batch).all()
    dstp = newpos[dst]

    core = dstp // NPC
    local = dstp - core * NPC
    blk = local >> 7
    dl = local & 127

    seg = core * NBLK + blk
    n_seg = NC * NBLK
    order = np.lexsort((src, seg))
    seg_o = seg[order]
    seg_cnt = np.bincount(seg_o, minlength=n_seg)
    C = int(np.max(seg_cnt))
    C = max(128, -(-C // 128) * 128)
    cfg.C = C
    EP = cfg.EP

    seg_start = np.zeros(n_seg, np.int64)
    np.cumsum(seg_cnt[:-1], out=seg_start[1:])
    within = np.arange(len(order)) - seg_start[seg_o]
    pos = (seg_o % NBLK) * C + within          # core-relative padded pos
    core_o = seg_o // NBLK

    src_at = np.zeros((NC, EP), np.int64)
    src_at[core_o, pos] = src[order]
    dstl_at = np.full((NC, EP), 128.0, np.float32)
    dstl_at[core_o, pos] = dl[order].astype(np.float32)
    ea_at = np.zeros((NC, EP, cfg.FE), np.float32)
    ea_at[core_o, pos] = edge_attr[order]

    # eaT4: 4-phase layout. Edge position p (chunk c=p//128, lane e=p%128)
    # maps to [32*(c%4)+f, (c//4)*128+e] — each 128-col block is a shared
    # K=128 matmul lhsT covering 4 chunks (phase selection via the
    # block-diagonal We).
    G4 = EP // 512
    eaT4 = ea_at.reshape(NC, G4, 4, 128, cfg.FE).transpose(0, 2, 4, 1, 3) \
        .reshape(NC, 4 * cfg.FE, G4 * 128).astype(BF16)

    dstcol = dstl_at.reshape(NC, EP // 128, 128).transpose(0, 2, 1) \
        .astype(BF16).copy()

    # node-side arrays (new node order)
    xT = x[invp].reshape(NC, NPC, H).transpose(0, 2, 1) \
        .astype(np.float32).copy()

    f32 = lambda a: np.asarray(a, np.float32)
    xtab = (x + f32(be1)[None, :]).astype(F8)

    def bnfold(g, bt, m, v, b):
        A_ = f32(g) / np.sqrt(f32(v) + cfg.bn_eps)
        B_ = A_ * f32(b) + (f32(bt) - A_ * f32(m))
        return A_.reshape(-1, 1), B_.reshape(-1, 1)

    A1, B1 = bnfold(g1, bt1, m1, v1, b1)
    A2, B2 = bnfold(g2, bt2, m2, v2, b2)
    Aa1, Ba1 = bnfold(ga1, bta1, ma1, va1, ba1)
    Aa2, Ba2 = bnfold(ga2, bta2, ma2, va2, ba2)

    def wsel(We_):  # [128, 4*H]: block q has We at rows 32q..32q+31
        W_ = np.zeros((128, 4 * H), np.float32)
        for q in range(4):
            W_[32 * q:32 * q + cfg.FE, q * H:(q + 1) * H] = f32(We_)
        return W_.astype(BF16)

    wts = dict(
        We1=wsel(We1),
        We2=wsel(We2),
        W1=f32(W1).astype(BF16), W2=f32(W2).astype(BF16),
        A1=A1, B1=B1, A2=A2, B2=B2,
        be2=f32(be2),
        # mean pool (1/1024) folded into Wa1
        Wa1=f32(Wa1) / (cfg.N // cfg.NG), Aa1=Aa1, Ba1=Ba1,
        Wa2=f32(Wa2), Aa2=Aa2, Ba2=Ba2,
        Wa3=f32(Wa3), ba3=f32(ba3).reshape(-1, 1),
    )
    percore = dict(eaT4=eaT4, dstcol=dstcol, xT=xT, src_at=src_at,
                   newpos=newpos)
    return xtab, percore, wts


def pack_stream(tab, src_at, EP):
    """tab [N, 128] bf16, src_at [NC, EP] -> [NC, 128, EP] bf16 where
    out[c, lane, ch*128+f] = tab[src_at[c, ch*128+lane], f]."""
    NC = src_at.shape[0]
    g = tab[src_at.reshape(-1)]                    # [NC*EP, 128]
    g = g.reshape(NC, EP // 128, 128, 128)         # [c, ch, lane, f]
    return np.ascontiguousarray(g.transpose(0, 2, 1, 3)).reshape(NC, 128, EP)


# --------------------------------------------------------- bass programs ----

def build_program(cfg, launch):
    """launch: 1 (conv1 -> h) or 2 (conv2 + pool + head)."""
    import concourse.bacc as bacc
    import concourse.tile as tile
    from concourse import mybir
    from concourse.masks import make_identity

    dt = mybir.dt
    AF = mybir.ActivationFunctionType
    OP = mybir.AluOpType
    H = cfg.H
    NPC, NBLK, WBLK, NW = cfg.NPC, cfg.NBLK, cfg.WBLK, cfg.NW
    C, CPB, CPW, EPW, EP = cfg.C, cfg.CPB, cfg.CPW, cfg.EPW, cfg.EP
    assert CPW % 4 == 0
    NG4 = CPW // 4
    # batches of 1-2 four-chunk groups sharing one PSUM tile / ACT
    batches = [(2 * i, 2 * i + 1) for i in range(NG4 // 2)]
    if NG4 % 2:
        batches.append((NG4 - 1,))

    nc = bacc.Bacc("TRN2", target_bir_lowering=False, debug=False,
                   enable_asserts=False, num_devices=cfg.n_cores)

    din = lambda n, s, d: nc.dram_tensor(n, s, d, kind="ExternalInput").ap()
    dout = lambda n, s, d: nc.dram_tensor(n, s, d, kind="ExternalOutput").ap()

    EPC = EP // 128
    CB16 = 4 * H + H + EPC           # We | W | dstcol
    CF32 = 2 if launch == 1 else 2 + H + 2 + H + 2 + cfg.A + 1
    xg = din("xg", [128, EP], dt.float8e4)
    eaT4 = din("eaT4", [128, EP // 4], dt.bfloat16)
    cb16 = din("cb16", [128, CB16], dt.bfloat16)
    cf32 = din("cf32", [128, CF32], dt.float32)
    if launch == 1:
        xT = din("xT", [128, NPC], dt.float32)
        hT_out = dout("hT_out", [128, NPC], dt.bfloat16)
    else:
        xT = din("xT", [128, NPC], dt.bfloat16)
        act_out = dout("act_out", [cfg.A, cfg.GPC], dt.float32)

    with tile.TileContext(nc) as tc:
        with (
            tc.tile_pool(name="const", bufs=1) as cpool,
            tc.tile_pool(name="xg", bufs=3) as xgpool,
            tc.tile_pool(name="stream", bufs=3) as spool,
            tc.tile_pool(name="sS", bufs=2) as spool_S,
            tc.tile_pool(name="work", bufs=3) as wpool,
            tc.tile_pool(name="blk", bufs=3) as bpool,
            tc.tile_pool(name="ps_t", bufs=2, space="PSUM") as ps_t,
            tc.tile_pool(name="ps_agg", bufs=2, space="PSUM") as ps_agg,
            tc.tile_pool(name="ps_misc", bufs=2, space="PSUM") as ps_misc,
        ):
            # ---- persistent constants: two blob DMAs, views by column slice
            cb16_sb = cpool.tile([128, CB16], dt.bfloat16, tag="cb16")
            cf32_sb = cpool.tile([128, CF32], dt.float32, tag="cf32")
            nc.sync.dma_start(cb16_sb[:], cb16[:])
            nc.sync.dma_start(cf32_sb[:], cf32[:])
            We_sb = cb16_sb[:, 0:4 * H]
            W_sb = cb16_sb[:, 4 * H:5 * H]
            dstcol_sb = cb16_sb[:, 5 * H:5 * H + EPC]
            A_sb = cf32_sb[:, 0:1]
            B_sb = cf32_sb[:, 1:2]

            iota_sb = cpool.tile([128, 128], dt.bfloat16, tag="iota")
            nc.gpsimd.iota(iota_sb[:], pattern=[[1, 128]], base=0,
                           channel_multiplier=0,
                           allow_small_or_imprecise_dtypes=True)
            id_f8 = cpool.tile([128, 128], dt.float8e4, tag="idf8")
            make_identity(nc, id_f8[:])

            if launch == 2:
                o = 2
                Wa1_sb = cf32_sb[:, o:o + H]; o += H
                Aa1_sb = cf32_sb[:, o:o + 1]; o += 1
                Ba1_sb = cf32_sb[:, o:o + 1]; o += 1
                Wa2_sb = cf32_sb[:, o:o + H]; o += H
                Aa2_sb = cf32_sb[:, o:o + 1]; o += 1
                Ba2_sb = cf32_sb[:, o:o + 1]; o += 1
                Wa3_sb = cf32_sb[:, o:o + cfg.A]; o += cfg.A
                ba3_sb = cf32_sb[0:cfg.A, o:o + 1]; o += 1
                bs_sb = cpool.tile([128, NBLK], dt.float32, tag="bs")

            # ---- main loop over windows (software-pipelined: each batch's
            # agg matmuls are emitted after the NEXT batch's t matmuls so
            # TensorE never head-of-line blocks on the ACT relu; each
            # window's drain is emitted after the next window's first batch)
            def emit_agg(msg, grp, S_list, agg_ps):
                for j in range(4 * len(grp)):
                    ch = grp[0] * 4 + j
                    bw, ci = divmod(ch, CPB)
                    nc.tensor.matmul(
                        agg_ps[:, bw * 128:(bw + 1) * 128],
                        lhsT=msg[:, j * 128:(j + 1) * 128],
                        rhs=S_list[bw][:, ci, :],
                        start=(ci == 0), stop=(ci == CPB - 1),
                        skip_group_check=True)

            def emit_drain(wdx, agg_ps, xt_sl):
                yT = wpool.tile([128, WBLK * 128], dt.bfloat16, tag="yT")
                nc.vector.tensor_tensor(out=yT[:], in0=agg_ps[:],
                                        in1=xt_sl[:], op=OP.add)
                if launch == 1:
                    hTw = bpool.tile([128, WBLK * 128], dt.bfloat16,
                                     tag="hTw", bufs=2)
                for k in range(WBLK):
                    b_abs = wdx * WBLK + k
                    hp_ps = ps_misc.tile([128, 128], dt.float32, tag="m")
                    nc.tensor.matmul(hp_ps[:], lhsT=W_sb,
                                     rhs=yT[:, k * 128:(k + 1) * 128],
                                     start=True, stop=True,
                                     skip_group_check=True)
                    if launch == 1:
                        nc.scalar.activation(hTw[:, k * 128:(k + 1) * 128],
                                             hp_ps[:], AF.Relu,
                                             bias=B_sb, scale=A_sb)
                    else:
                        # sigmoid(relu(z)) == max(sigmoid(z), 0.5)
                        sT = bpool.tile([128, 128], dt.float32, tag="sT")
                        nc.scalar.activation(sT[:], hp_ps[:], AF.Sigmoid,
                                             bias=B_sb, scale=A_sb)
                        h2T = bpool.tile([128, 128], dt.bfloat16, tag="h2T")
                        nc.vector.tensor_scalar(
                            out=h2T[:], in0=sT[:], scalar1=0.5, scalar2=0.0,
                            op0=OP.max, op1=OP.add,
                            accum_out=bs_sb[:, b_abs:b_abs + 1])
                if launch == 1:
                    nc.sync.dma_start(
                        hT_out[:, wdx * WBLK * 128:(wdx + 1) * WBLK * 128],
                        hTw[:])

            pend_agg = None      # (msg, grp, S_list, agg_ps)
            pend_drain = []      # [slots_left, (wdx, agg_ps, xt_sl)]
            for wdx in range(NW):
                xg_sl = xgpool.tile([128, EPW], dt.float8e4, tag="xg")
                nc.sync.dma_start(xg_sl[:],
                                  xg[:, wdx * EPW:(wdx + 1) * EPW])
                ea_sl = spool.tile([128, EPW // 4], dt.bfloat16, tag="ea")
                nc.sync.dma_start(
                    ea_sl[:], eaT4[:, wdx * (EPW // 4):(wdx + 1) * (EPW // 4)])
                xt_sl = spool.tile([128, WBLK * 128],
                                   dt.float32 if launch == 1 else dt.bfloat16,
                                   tag="xt", bufs=3)
                nc.sync.dma_start(xt_sl[:],
                                  xT[:, wdx * WBLK * 128:(wdx + 1) * WBLK * 128])

                # dst one-hot S per 128-dst block (CPB chunks each)
                S_blk = []
                for bw in range(WBLK):
                    c0 = wdx * CPW + bw * CPB
                    S_b = spool_S.tile([128, CPB, 128], dt.bfloat16,
                                       tag=f"S{bw}")
                    iota_b = iota_sb[:].unsqueeze(1) \
                        .to_broadcast([128, CPB, 128])
                    dst_b = dstcol_sb[:, c0:c0 + CPB].unsqueeze(2) \
                        .to_broadcast([128, CPB, 128])
                    if os.environ.get("BASS_GNN_GPS_S") and bw % 2 == 1:
                        nc.gpsimd.scalar_tensor_tensor(
                            out=S_b[:], in0=dst_b, scalar=0.0, in1=iota_b,
                            op0=OP.add, op1=OP.is_equal)
                    else:
                        nc.vector.tensor_tensor(
                            out=S_b[:], in0=iota_b, in1=dst_b,
                            op=OP.is_equal)
                    S_blk.append(S_b)

                agg_ps = ps_agg.tile([128, WBLK * 128], dt.float32, tag="agg")

                for grp in batches:
                    nw = 512 * len(grp)
                    t_ps = ps_t.tile([128, 1024], dt.float32, tag="t")
                    for gi, Gw in enumerate(grp):
                        lhs = ea_sl[:, Gw * 128:(Gw + 1) * 128]
                        nc.tensor.matmul(t_ps[:, gi * 512:(gi + 1) * 512],
                                         lhsT=lhs, rhs=We_sb,
                                         start=True, stop=False,
                                         skip_group_check=True)
                    for gi, Gw in enumerate(grp):
                        nc.tensor.matmul(t_ps[:, gi * 512:(gi + 1) * 512],
                                         lhsT=id_f8[:],
                                         rhs=xg_sl[:, Gw * 512:(Gw + 1) * 512],
                                         start=False, stop=True,
                                         skip_group_check=True)
                    msg = wpool.tile([128, 1024], dt.bfloat16, tag="msg")
                    nc.scalar.activation(msg[:, 0:nw], t_ps[:, 0:nw], AF.Relu)
                    if pend_agg is not None:
                        emit_agg(*pend_agg)
                    for d in pend_drain:
                        d[0] -= 1
                    if pend_drain and pend_drain[0][0] <= 0:
                        emit_drain(*pend_drain.pop(0)[1])
                    pend_agg = (msg, grp, S_blk, agg_ps)
                pend_drain.append([2, (wdx, agg_ps, xt_sl)])

            emit_agg(*pend_agg)
            for _, args in pend_drain:
                emit_drain(*args)

            if launch == 2:
                # per-graph sums (graphs are 8 consecutive blocks), head
                pooledT = bpool.tile([128, cfg.GPC], dt.float32, tag="plT")
                for g in range(cfg.GPC):
                    nc.vector.tensor_reduce(
                        out=pooledT[:, g:g + 1],
                        in_=bs_sb[:, g * 8:(g + 1) * 8],
                        axis=mybir.AxisListType.X, op=OP.add)

                a1_ps = ps_misc.tile([128, cfg.GPC], dt.float32, tag="m")
                nc.tensor.matmul(a1_ps[:], lhsT=Wa1_sb, rhs=pooledT[:],
                                 start=True, stop=True, skip_group_check=True)
                a1 = bpool.tile([128, cfg.GPC], dt.float32, tag="a1")
                nc.scalar.activation(a1[:], a1_ps[:], AF.Relu,
                                     bias=Ba1_sb, scale=Aa1_sb)
                a2_ps = ps_misc.tile([128, cfg.GPC], dt.float32, tag="m")
                nc.tensor.matmul(a2_ps[:], lhsT=Wa2_sb, rhs=a1[:],
                                 start=True, stop=True, skip_group_check=True)
                a2 = bpool.tile([128, cfg.GPC], dt.float32, tag="a2")
                nc.scalar.activation(a2[:], a2_ps[:], AF.Relu,
                                     bias=Ba2_sb, scale=Aa2_sb)
                a3_ps = ps_misc.tile([cfg.A, cfg.GPC], dt.float32, tag="m")
                nc.tensor.matmul(a3_ps[:], lhsT=Wa3_sb, rhs=a2[:],
                                 start=True, stop=True, skip_group_check=True)
                a3 = bpool.tile([cfg.A, cfg.GPC], dt.float32, tag="a3")
                nc.scalar.activation(a3[:], a3_ps[:], AF.Sigmoid,
                                     bias=ba3_sb)
                nc.sync.dma_start(act_out[:], a3[:])

    nc.compile()
    return nc


# ------------------------------------------------------------- execution ----

def make_in_maps(cfg, launch, xg_pc, percore, wts, hT_percore=None):
    NC = cfg.n_cores
    f32 = np.float32
    if launch == 1:
        cf32_shared = np.concatenate([wts["A1"], wts["B1"]], axis=1) \
            .astype(f32)
        We_, W_ = wts["We1"], wts["W1"]
    else:
        ba3p = np.zeros((128, 1), f32)
        ba3p[:cfg.A] = wts["ba3"]
        cf32_shared = np.concatenate(
            [wts["A2"], wts["B2"], wts["Wa1"], wts["Aa1"], wts["Ba1"],
             wts["Wa2"], wts["Aa2"], wts["Ba2"], wts["Wa3"], ba3p],
            axis=1).astype(f32)
        We_, W_ = wts["We2"], wts["W2"]
    maps = []
    for c in range(NC):
        cb16 = np.concatenate(
            [We_, W_, percore["dstcol"][c]], axis=1).astype(BF16)
        m = dict(xg=np.ascontiguousarray(xg_pc[c]),
                 eaT4=np.ascontiguousarray(percore["eaT4"][c]),
                 cb16=np.ascontiguousarray(cb16),
                 cf32=np.ascontiguousarray(cf32_shared))
        if launch == 1:
            m.update(xT=np.ascontiguousarray(percore["xT"][c]))
        else:
            m.update(xT=np.ascontiguousarray(hT_percore[c]))
        maps.append(m)
    return maps


_PROG_CACHE = {}
LAST_EXEC_NS = {}


def kernel(**inputs):
    from concourse import bass_utils
    _enable_ldw_opt()

    cfg = Cfg()
    xtab, percore, wts = host_prep(cfg, **inputs)

    key = (cfg.N, cfg.E, cfg.C)
    if key not in _PROG_CACHE:
        _PROG_CACHE[key] = (build_program(cfg, 1), build_program(cfg, 2))
    nc1, nc2 = _PROG_CACHE[key]

    trace = bool(int(os.environ.get("BASS_GNN_TRACE", "0")))
    core_ids = list(range(cfg.n_cores))

    xg1 = pack_stream(xtab, percore["src_at"], cfg.EP)
    maps1 = make_in_maps(cfg, 1, xg1, percore, wts)
    res1 = bass_utils.run_bass_kernel_spmd(nc1, maps1, core_ids=core_ids,
                                           trace=trace)
    LAST_EXEC_NS["L1"] = res1.exec_time_ns
    if os.environ.get("BASS_GNN_ONLY_L1"):
        return res1
    hT = [res1.results[c]["hT_out"] for c in core_ids]      # [128, NPC] bf16

    h_all = np.concatenate([t.T for t in hT], axis=0)       # [N, H] new order
    h_orig = h_all[percore["newpos"]]                       # rows by orig id
    htab = (h_orig.astype(np.float32) + wts["be2"][None, :]).astype(F8)
    xg2 = pack_stream(htab, percore["src_at"], cfg.EP)

    maps2 = make_in_maps(cfg, 2, xg2, percore, wts, hT_percore=hT)
    res2 = bass_utils.run_bass_kernel_spmd(nc2, maps2, core_ids=core_ids,
                                           trace=trace)
    LAST_EXEC_NS["L2"] = res2.exec_time_ns

    out = np.zeros((cfg.NG, cfg.A), np.float32)
    for c in core_ids:
        a3 = res2.results[c]["act_out"]          # [A, GPC]
        out[c * cfg.GPC:(c + 1) * cfg.GPC, :] = a3.T
    return out


# revision 44
# speedup vs baseline: 1.1197x; 1.0285x over previous
"""Trainium2 Bass kernel for nn_ActionModel (2x GINEConv + mean-pool + MLP head).

Strategy (8 NeuronCores, SPMD):
  - Nodes sharded by graph: core m owns 8 consecutive graphs = 8192 nodes.
  - Edges sharded by dst owner; per core, edges are grouped by 128-dst block,
    padded to a fixed per-block capacity C so the instruction stream is
    identical across cores.
  - Host prep builds, per core, sequentially-streamable operand arrays in
    padded edge order (the same treatment the edge_attr already gets):
      * xg  : x[src]+be (bf16) laid out [128 lanes, chunk, feat]
      * eaT4: edge_attr 4-phase packed so one K=128 matmul against a
              block-diagonal We computes ea@We for 4 chunks at once
      * dstcol: per-edge dst-local-in-block (bf16, 128 = padding sentinel)
  - On-device, per 1024-edge pair of 4-chunk groups:
      TensorE: ea@We (one N=512 matmul per group) + identity-matmul add of
      xg into PSUM; ACT applies ReLU over [128,1024] -> bf16 msg; DVE builds
      the dst one-hot S per 128-dst block (iota/is_equal); TensorE
      accumulates aggT += msg^T @ S into [feat, dst] PSUM.
  - Node stage: yT = aggT + xT; Linear+folded-BN+ReLU via TensorE/ACT.
  - Two launches: L1 -> hT (bf16); host rebuilds the conv2 edge stream
    (h+be2)[src]; L2 runs conv2, sigmoid with per-block accum_out giving
    block sums, per-graph mean pool (graphs are contiguous 1024-node
    ranges), and the 3-layer head. Only [A, GPC] per core comes back.
"""

import heapq
import os
import sys
import numpy as np

for _p in ("/opt/trn_rl_repo",):
    if _p not in sys.path and os.path.isdir(_p):
        sys.path.insert(0, _p)

import ml_dtypes  # noqa: E402

BF16 = ml_dtypes.bfloat16
F8 = ml_dtypes.float8_e4m3


def _enable_ldw_opt():
    """Flip walrus's --enable-ldw-opt to true (merges/accelerates redundant
    LDWEIGHTS). Wraps concourse.bass_utils.run_command."""
    # walrus rejects bass-emitted InstLdweights under ldw-opt; keep off
    # unless explicitly requested for experiments.
    if not os.environ.get("BASS_GNN_LDWOPT"):
        return
    from concourse import bass_utils as _bu
    if getattr(_bu, "_gnn_ldwopt_patched", False):
        return
    _orig = _bu.run_command

    def _patched(cmd, *a, **k):
        if isinstance(cmd, list):
            cmd = ["--enable-ldw-opt=true" if c == "--enable-ldw-opt=false"
                   else c for c in cmd]
        return _orig(cmd, *a, **k)

    _bu.run_command = _patched
    _bu._gnn_ldwopt_patched = True

# ---------------------------------------------------------------- config ----

class Cfg:
    def __init__(self, N=65536, E=1048576, H=128, FE=32, NG=64, A=32,
                 n_cores=8, WBLK=4, bn_eps=1e-5):
        self.N, self.E, self.H, self.FE, self.NG, self.A = N, E, H, FE, NG, A
        self.n_cores = n_cores
        self.WBLK = WBLK          # dst blocks per window
        self.bn_eps = bn_eps
        self.NPC = N // n_cores   # nodes per core
        self.GPC = NG // n_cores  # graphs per core
        self.NBLK = self.NPC // 128
        assert self.NPC % 128 == 0 and self.NBLK % WBLK == 0
        self.NW = self.NBLK // WBLK
        self.C = None             # per-block capacity; set by prep

    @property
    def CPB(self):  # chunks per block
        return self.C // 128

    @property
    def CPW(self):  # chunks per window
        return self.WBLK * self.CPB

    @property
    def EPW(self):  # padded edge positions per window
        return self.CPW * 128

    @property
    def EP(self):   # padded edge positions per core
        return self.NBLK * self.C


# ------------------------------------------------------------- host prep ----

def host_prep(cfg, x, edge_index, edge_attr, batch,
              We1, be1, W1, b1, g1, bt1, m1, v1,
              We2, be2, W2, b2, g2, bt2, m2, v2,
              Wa1, ba1, ga1, bta1, ma1, va1,
              Wa2, ba2, ga2, bta2, ma2, va2,
              Wa3, ba3):
    """Partition/sort/pad edges, build per-core streamable arrays."""
    N, H, NC = cfg.N, cfg.H, cfg.n_cores
    NPC, NBLK = cfg.NPC, cfg.NBLK

    src = np.asarray(edge_index[0], dtype=np.int64)
    dst = np.asarray(edge_index[1], dtype=np.int64)
    batch = np.asarray(batch, dtype=np.int64)
    x = np.asarray(x, dtype=np.float32)
    edge_attr = np.asarray(edge_attr, dtype=np.float32)

    cnts = np.bincount(batch, minlength=cfg.NG)
    assert (cnts == cfg.N // cfg.NG).all(), "equal-size graphs expected"

    # Within-graph node relabeling balancing per-block in-degree (greedy
    # first-fit-decreasing into the 8 blocks of each graph). Shrinks the
    # padded per-block capacity C. Pooling is within-graph permutation
    # invariant; the gather table stays in original node ids.
    GS = N // cfg.NG
    BPG = GS // 128
    indeg = np.bincount(dst, minlength=N)
    newpos = np.empty(N, np.int64)
    for g in range(cfg.NG):
        deg = indeg[g * GS:(g + 1) * GS]
        order_g = np.argsort(-deg, kind="stable")
        heap = [(0, 0, b) for b in range(BPG)]
        heapq.heapify(heap)
        slot = np.empty(GS, np.int64)
        for nd in order_g:
            load, c, b = heapq.heappop(heap)
            slot[nd] = b * 128 + c
            load += int(deg[nd])
            c += 1
            if c < 128:
                heapq.heappush(heap, (load, c, b))
        newpos[g * GS:(g + 1) * GS] = g * GS + slot
    invp = np.argsort(newpos)
    assert (batch[invp] == batch).all()
    dstp = newpos[dst]

    core = dstp // NPC
    local = dstp - core * NPC
    blk = local >> 7
    dl = local & 127

    seg = core * NBLK + blk
    n_seg = NC * NBLK
    order = np.lexsort((src, seg))
    seg_o = seg[order]
    seg_cnt = np.bincount(seg_o, minlength=n_seg)
    C = int(np.max(seg_cnt))
    C = max(128, -(-C // 128) * 128)
    cfg.C = C
    EP = cfg.EP

    seg_start = np.zeros(n_seg, np.int64)
    np.cumsum(seg_cnt[:-1], out=seg_start[1:])
    within = np.arange(len(order)) - seg_start[seg_o]
    pos = (seg_o % NBLK) * C + within          # core-relative padded pos
    core_o = seg_o // NBLK

    src_at = np.zeros((NC, EP), np.int64)
    src_at[core_o, pos] = src[order]
    dstl_at = np.full((NC, EP), 128.0, np.float32)
    dstl_at[core_o, pos] = dl[order].astype(np.float32)
    ea_at = np.zeros((NC, EP, cfg.FE), np.float32)
    ea_at[core_o, pos] = edge_attr[order]

    # eaT4: 4-phase layout. Edge position p (chunk c=p//128, lane e=p%128)
    # maps to [32*(c%4)+f, (c//4)*128+e] — each 128-col block is a shared
    # K=128 matmul lhsT covering 4 chunks (phase selection via the
    # block-diagonal We).
    G4 = EP // 512
    eaT4 = ea_at.reshape(NC, G4, 4, 128, cfg.FE).transpose(0, 2, 4, 1, 3) \
        .reshape(NC, 4 * cfg.FE, G4 * 128).astype(BF16)

    dstcol = dstl_at.reshape(NC, EP // 128, 128).transpose(0, 2, 1) \
        .astype(BF16).copy()

    # node-side arrays (new node order)
    xT = x[invp].reshape(NC, NPC, H).transpose(0, 2, 1) \
        .astype(np.float32).copy()

    f32 = lambda a: np.asarray(a, np.float32)
    xtab = (x + f32(be1)[None, :]).astype(F8)

    def bnfold(g, bt, m, v, b):
        A_ = f32(g) / np.sqrt(f32(v) + cfg.bn_eps)
        B_ = A_ * f32(b) + (f32(bt) - A_ * f32(m))
        return A_.reshape(-1, 1), B_.reshape(-1, 1)

    A1, B1 = bnfold(g1, bt1, m1, v1, b1)
    A2, B2 = bnfold(g2, bt2, m2, v2, b2)
    Aa1, Ba1 = bnfold(ga1, bta1, ma1, va1, ba1)
    Aa2, Ba2 = bnfold(ga2, bta2, ma2, va2, ba2)

    def wsel(We_):  # [128, 4*H]: block q has We at rows 32q..32q+31
        W_ = np.zeros((128, 4 * H), np.float32)
        for q in range(4):
            W_[32 * q:32 * q + cfg.FE, q * H:(q + 1) * H] = f32(We_)
        return W_.astype(BF16)

    wts = dict(
        We1=wsel(We1),
        We2=wsel(We2),
        W1=f32(W1).astype(BF16), W2=f32(W2).astype(BF16),
        A1=A1, B1=B1, A2=A2, B2=B2,
        be2=f32(be2),
        # mean pool (1/1024) folded into Wa1
        Wa1=f32(Wa1) / (cfg.N // cfg.NG), Aa1=Aa1, Ba1=Ba1,
        Wa2=f32(Wa2), Aa2=Aa2, Ba2=Ba2,
        Wa3=f32(Wa3), ba3=f32(ba3).reshape(-1, 1),
    )
    percore = dict(eaT4=eaT4, dstcol=dstcol, xT=xT, src_at=src_at,
                   newpos=newpos)
    return xtab, percore, wts


def pack_stream(tab, src_at, EP):
    """tab [N, 128] bf16, src_at [NC, EP] -> [NC, 128, EP] bf16 where
    out[c, lane, ch*128+f] = tab[src_at[c, ch*128+lane], f]."""
    NC = src_at.shape[0]
    g = tab[src_at.reshape(-1)]                    # [NC*EP, 128]
    g = g.reshape(NC, EP // 128, 128, 128)         # [c, ch, lane, f]
    return np.ascontiguousarray(g.transpose(0, 2, 1, 3)).reshape(NC, 128, EP)


# --------------------------------------------------------- bass programs ----

def build_program(cfg, launch):
    """launch: 1 (conv1 -> h) or 2 (conv2 + pool + head)."""
    import concourse.bacc as bacc
    import concourse.tile as tile
    from concourse import mybir
    from concourse.masks import make_identity

    dt = mybir.dt
    AF = mybir.ActivationFunctionType
    OP = mybir.AluOpType
    H = cfg.H
    NPC, NBLK, WBLK, NW = cfg.NPC, cfg.NBLK, cfg.WBLK, cfg.NW
    C, CPB, CPW, EPW, EP = cfg.C, cfg.CPB, cfg.CPW, cfg.EPW, cfg.EP
    assert CPW % 4 == 0
    NG4 = CPW // 4
    # batches of 1-2 four-chunk groups sharing one PSUM tile / ACT
    batches = [(2 * i, 2 * i + 1) for i in range(NG4 // 2)]
    if NG4 % 2:
        batches.append((NG4 - 1,))

    nc = bacc.Bacc("TRN2", target_bir_lowering=False, debug=False,
                   enable_asserts=False, num_devices=cfg.n_cores)

    din = lambda n, s, d: nc.dram_tensor(n, s, d, kind="ExternalInput").ap()
    dout = lambda n, s, d: nc.dram_tensor(n, s, d, kind="ExternalOutput").ap()

    EPC = EP // 128
    CB16 = 4 * H + H + EPC           # We | W | dstcol
    CF32 = 2 if launch == 1 else 2 + H + 2 + H + 2 + cfg.A + 1
    xg = din("xg", [128, EP], dt.float8e4)
    eaT4 = din("eaT4", [128, EP // 4], dt.bfloat16)
    cb16 = din("cb16", [128, CB16], dt.bfloat16)
    cf32 = din("cf32", [128, CF32], dt.float32)
    if launch == 1:
        xT = din("xT", [128, NPC], dt.float32)
        hT_out = dout("hT_out", [128, NPC], dt.bfloat16)
    else:
        xT = din("xT", [128, NPC], dt.bfloat16)
        act_out = dout("act_out", [cfg.A, cfg.GPC], dt.float32)

    with tile.TileContext(nc) as tc:
        with (
            tc.tile_pool(name="const", bufs=1) as cpool,
            tc.tile_pool(name="xg", bufs=3) as xgpool,
            tc.tile_pool(name="stream", bufs=3) as spool,
            tc.tile_pool(name="sS", bufs=2) as spool_S,
            tc.tile_pool(name="work", bufs=3) as wpool,
            tc.tile_pool(name="blk", bufs=3) as bpool,
            tc.tile_pool(name="ps_t", bufs=2, space="PSUM") as ps_t,
            tc.tile_pool(name="ps_agg", bufs=2, space="PSUM") as ps_agg,
            tc.tile_pool(name="ps_misc", bufs=2, space="PSUM") as ps_misc,
        ):
            # ---- persistent constants: two blob DMAs, views by column slice
            cb16_sb = cpool.tile([128, CB16], dt.bfloat16, tag="cb16")
            cf32_sb = cpool.tile([128, CF32], dt.float32, tag="cf32")
            nc.sync.dma_start(cb16_sb[:], cb16[:])
            nc.sync.dma_start(cf32_sb[:], cf32[:])
            We_sb = cb16_sb[:, 0:4 * H]
            W_sb = cb16_sb[:, 4 * H:5 * H]
            dstcol_sb = cb16_sb[:, 5 * H:5 * H + EPC]
            A_sb = cf32_sb[:, 0:1]
            B_sb = cf32_sb[:, 1:2]

            iota_sb = cpool.tile([128, 128], dt.bfloat16, tag="iota")
            nc.gpsimd.iota(iota_sb[:], pattern=[[1, 128]], base=0,
                           channel_multiplier=0,
                           allow_small_or_imprecise_dtypes=True)
            id_f8 = cpool.tile([128, 128], dt.float8e4, tag="idf8")
            make_identity(nc, id_f8[:])

            if launch == 2:
                o = 2
                Wa1_sb = cf32_sb[:, o:o + H]; o += H
                Aa1_sb = cf32_sb[:, o:o + 1]; o += 1
                Ba1_sb = cf32_sb[:, o:o + 1]; o += 1
                Wa2_sb = cf32_sb[:, o:o + H]; o += H
                Aa2_sb = cf32_sb[:, o:o + 1]; o += 1
                Ba2_sb = cf32_sb[:, o:o + 1]; o += 1
                Wa3_sb = cf32_sb[:, o:o + cfg.A]; o += cfg.A
                ba3_sb = cf32_sb[0:cfg.A, o:o + 1]; o += 1
                bs_sb = cpool.tile([128, NBLK], dt.float32, tag="bs")

            # ---- main loop over windows (software-pipelined: each batch's
            # agg matmuls are emitted after the NEXT batch's t matmuls so
            # TensorE never head-of-line blocks on the ACT relu; each
            # window's drain is emitted after the next window's first batch)
            def emit_agg(msg, grp, S_list, agg_ps):
                for j in range(4 * len(grp)):
                    ch = grp[0] * 4 + j
                    bw, ci = divmod(ch, CPB)
                    nc.tensor.matmul(
                        agg_ps[:, bw * 128:(bw + 1) * 128],
                        lhsT=msg[:, j * 128:(j + 1) * 128],
                        rhs=S_list[bw][:, ci, :],
                        start=(ci == 0), stop=(ci == CPB - 1),
                        skip_group_check=True)

            def emit_drain(wdx, agg_ps, xt_sl):
                yT = wpool.tile([128, WBLK * 128], dt.bfloat16, tag="yT")
                nc.vector.tensor_tensor(out=yT[:], in0=agg_ps[:],
                                        in1=xt_sl[:], op=OP.add)
                hp_ps = ps_misc.tile([128, WBLK * 128], dt.float32, tag="m")
                for k in range(WBLK):
                    nc.tensor.matmul(hp_ps[:, k * 128:(k + 1) * 128],
                                     lhsT=W_sb,
                                     rhs=yT[:, k * 128:(k + 1) * 128],
                                     start=True, stop=True,
                                     skip_group_check=True)
                if launch == 1:
                    hTw = bpool.tile([128, WBLK * 128], dt.bfloat16,
                                     tag="hTw", bufs=2)
                    nc.scalar.activation(hTw[:], hp_ps[:], AF.Relu,
                                         bias=B_sb, scale=A_sb)
                    nc.sync.dma_start(
                        hT_out[:, wdx * WBLK * 128:(wdx + 1) * WBLK * 128],
                        hTw[:])
                else:
                    # sigmoid(relu(z)) == max(sigmoid(z), 0.5)
                    sT = bpool.tile([128, WBLK * 128], dt.float32, tag="sT")
                    nc.scalar.activation(sT[:], hp_ps[:], AF.Sigmoid,
                                         bias=B_sb, scale=A_sb)
                    h2T = bpool.tile([128, WBLK * 128], dt.bfloat16,
                                     tag="h2T")
                    for k in range(WBLK):
                        b_abs = wdx * WBLK + k
                        nc.vector.tensor_scalar(
                            out=h2T[:, k * 128:(k + 1) * 128],
                            in0=sT[:, k * 128:(k + 1) * 128],
                            scalar1=0.5, scalar2=0.0,
                            op0=OP.max, op1=OP.add,
                            accum_out=bs_sb[:, b_abs:b_abs + 1])

            pend_agg = None      # (msg, grp, S_list, agg_ps)
            pend_drain = []      # [slots_left, (wdx, agg_ps, xt_sl)]
            for wdx in range(NW):
                xg_sl = xgpool.tile([128, EPW], dt.float8e4, tag="xg")
                nc.sync.dma_start(xg_sl[:],
                                  xg[:, wdx * EPW:(wdx + 1) * EPW])
                ea_sl = spool.tile([128, EPW // 4], dt.bfloat16, tag="ea")
                nc.sync.dma_start(
                    ea_sl[:], eaT4[:, wdx * (EPW // 4):(wdx + 1) * (EPW // 4)])
                xt_sl = spool.tile([128, WBLK * 128],
                                   dt.float32 if launch == 1 else dt.bfloat16,
                                   tag="xt", bufs=3)
                nc.sync.dma_start(xt_sl[:],
                                  xT[:, wdx * WBLK * 128:(wdx + 1) * WBLK * 128])

                # dst one-hot S per 128-dst block (CPB chunks each)
                S_blk = []
                for bw in range(WBLK):
                    c0 = wdx * CPW + bw * CPB
                    S_b = spool_S.tile([128, CPB, 128], dt.bfloat16,
                                       tag=f"S{bw}")
                    iota_b = iota_sb[:].unsqueeze(1) \
                        .to_broadcast([128, CPB, 128])
                    dst_b = dstcol_sb[:, c0:c0 + CPB].unsqueeze(2) \
                        .to_broadcast([128, CPB, 128])
                    if os.environ.get("BASS_GNN_GPS_S") and bw % 2 == 1:
                        nc.gpsimd.scalar_tensor_tensor(
                            out=S_b[:], in0=dst_b, scalar=0.0, in1=iota_b,
                            op0=OP.add, op1=OP.is_equal)
                    else:
                        nc.vector.tensor_tensor(
                            out=S_b[:], in0=iota_b, in1=dst_b,
                            op=OP.is_equal)
                    S_blk.append(S_b)

                agg_ps = ps_agg.tile([128, WBLK * 128], dt.float32, tag="agg")

                for grp in batches:
                    nw = 512 * len(grp)
                    t_ps = ps_t.tile([128, 1024], dt.float32, tag="t")
                    for gi, Gw in enumerate(grp):
                        lhs = ea_sl[:, Gw * 128:(Gw + 1) * 128]
                        nc.tensor.matmul(t_ps[:, gi * 512:(gi + 1) * 512],
                                         lhsT=lhs, rhs=We_sb,
                                         start=True, stop=False,
                                         skip_group_check=True)
                    for gi, Gw in enumerate(grp):
                        nc.tensor.matmul(t_ps[:, gi * 512:(gi + 1) * 512],
                                         lhsT=id_f8[:],
                                         rhs=xg_sl[:, Gw * 512:(Gw + 1) * 512],
                                         start=False, stop=True,
                                         skip_group_check=True)
                    msg = wpool.tile([128, 1024], dt.bfloat16, tag="msg")
                    nc.scalar.activation(msg[:, 0:nw], t_ps[:, 0:nw], AF.Relu)
                    if pend_agg is not None:
                        emit_agg(*pend_agg)
                    for d in pend_drain:
                        d[0] -= 1
                    if pend_drain and pend_drain[0][0] <= 0:
                        emit_drain(*pend_drain.pop(0)[1])
                    pend_agg = (msg, grp, S_blk, agg_ps)
                pend_drain.append([2, (wdx, agg_ps, xt_sl)])

            emit_agg(*pend_agg)
            for _, args in pend_drain:
                emit_drain(*args)

            if launch == 2:
                # per-graph sums (graphs are 8 consecutive blocks), head
                pooledT = bpool.tile([128, cfg.GPC], dt.float32, tag="plT")
                for g in range(cfg.GPC):
                    nc.vector.tensor_reduce(
                        out=pooledT[:, g:g + 1],
                        in_=bs_sb[:, g * 8:(g + 1) * 8],
                        axis=mybir.AxisListType.X, op=OP.add)

                a1_ps = ps_misc.tile([128, cfg.GPC], dt.float32, tag="m")
                nc.tensor.matmul(a1_ps[:], lhsT=Wa1_sb, rhs=pooledT[:],
                                 start=True, stop=True, skip_group_check=True)
                a1 = bpool.tile([128, cfg.GPC], dt.float32, tag="a1")
                nc.scalar.activation(a1[:], a1_ps[:], AF.Relu,
                                     bias=Ba1_sb, scale=Aa1_sb)
                a2_ps = ps_misc.tile([128, cfg.GPC], dt.float32, tag="m")
                nc.tensor.matmul(a2_ps[:], lhsT=Wa2_sb, rhs=a1[:],
                                 start=True, stop=True, skip_group_check=True)
                a2 = bpool.tile([128, cfg.GPC], dt.float32, tag="a2")
                nc.scalar.activation(a2[:], a2_ps[:], AF.Relu,
                                     bias=Ba2_sb, scale=Aa2_sb)
                a3_ps = ps_misc.tile([cfg.A, cfg.GPC], dt.float32, tag="m")
                nc.tensor.matmul(a3_ps[:], lhsT=Wa3_sb, rhs=a2[:],
                                 start=True, stop=True, skip_group_check=True)
                a3 = bpool.tile([cfg.A, cfg.GPC], dt.float32, tag="a3")
                nc.scalar.activation(a3[:], a3_ps[:], AF.Sigmoid,
                                     bias=ba3_sb)
                nc.sync.dma_start(act_out[:], a3[:])

    nc.compile()
    return nc


# ------------------------------------------------------------- execution ----

def make_in_maps(cfg, launch, xg_pc, percore, wts, hT_percore=None):
    NC = cfg.n_cores
    f32 = np.float32
    if launch == 1:
        cf32_shared = np.concatenate([wts["A1"], wts["B1"]], axis=1) \
            .astype(f32)
        We_, W_ = wts["We1"], wts["W1"]
    else:
        ba3p = np.zeros((128, 1), f32)
        ba3p[:cfg.A] = wts["ba3"]
        cf32_shared = np.concatenate(
            [wts["A2"], wts["B2"], wts["Wa1"], wts["Aa1"], wts["Ba1"],
             wts["Wa2"], wts["Aa2"], wts["Ba2"], wts["Wa3"], ba3p],
            axis=1).astype(f32)
        We_, W_ = wts["We2"], wts["W2"]
    maps = []
    for c in range(NC):
        cb16 = np.concatenate(
            [We_, W_, percore["dstcol"][c]], axis=1).astype(BF16)
        m = dict(xg=np.ascontiguousarray(xg_pc[c]),
                 eaT4=np.ascontiguousarray(percore["eaT4"][c]),
                 cb16=np.ascontiguousarray(cb16),
                 cf32=np.ascontiguousarray(cf32_shared))
        if launch == 1:
            m.update(xT=np.ascontiguousarray(percore["xT"][c]))
        else:
            m.update(xT=np.ascontiguousarray(hT_percore[c]))
        maps.append(m)
    return maps


_PROG_CACHE = {}
LAST_EXEC_NS = {}


def kernel(**inputs):
    from concourse import bass_utils
    _enable_ldw_opt()

    cfg = Cfg()
    xtab, percore, wts = host_prep(cfg, **inputs)

    key = (cfg.N, cfg.E, cfg.C)
    if key not in _PROG_CACHE:
        _PROG_CACHE[key] = (build_program(cfg, 1), build_program(cfg, 2))
    nc1, nc2 = _PROG_CACHE[key]

    trace = bool(int(os.environ.get("BASS_GNN_TRACE", "0")))
    core_ids = list(range(cfg.n_cores))

    xg1 = pack_stream(xtab, percore["src_at"], cfg.EP)
    maps1 = make_in_maps(cfg, 1, xg1, percore, wts)
    res1 = bass_utils.run_bass_kernel_spmd(nc1, maps1, core_ids=core_ids,
                                           trace=trace)
    LAST_EXEC_NS["L1"] = res1.exec_time_ns
    if os.environ.get("BASS_GNN_ONLY_L1"):
        return res1
    hT = [res1.results[c]["hT_out"] for c in core_ids]      # [128, NPC] bf16

    h_all = np.concatenate([t.T for t in hT], axis=0)       # [N, H] new order
    h_orig = h_all[percore["newpos"]]                       # rows by orig id
    htab = (h_orig.astype(np.float32) + wts["be2"][None, :]).astype(F8)
    xg2 = pack_stream(htab, percore["src_at"], cfg.EP)

    maps2 = make_in_maps(cfg, 2, xg2, percore, wts, hT_percore=hT)
    res2 = bass_utils.run_bass_kernel_spmd(nc2, maps2, core_ids=core_ids,
                                           trace=trace)
    LAST_EXEC_NS["L2"] = res2.exec_time_ns

    out = np.zeros((cfg.NG, cfg.A), np.float32)
    for c in core_ids:
        a3 = res2.results[c]["act_out"]          # [A, GPC]
        out[c * cfg.GPC:(c + 1) * cfg.GPC, :] = a3.T
    return out
